# revision 64
# baseline (speedup 1.0000x reference)
"""MemMambaBlock Trainium2 kernel (self-contained).

8-core SPMD: head-sharded in_proj/conv/SSD -> AllToAll -> token-sharded
norm/out_proj/scorer/top-50-pool/retrieval/gate. The sequential memory-pool
scan is replaced by an exact parallel top-50 selection (streaming top-k
equivalence; slot order is irrelevant because the masked softmax retrieval
is permutation-invariant over pool slots).

v1: bf16 weights/activations, prefetched tail weights, PE-broadcast decay
matrices (no vector scans), vector/gpsimd split, per-chunk A2A staging.
"""
import contextlib
import numpy as np
import concourse.bass as bass
import concourse.bacc as bacc
import concourse.mybir as mybir
import concourse.tile as tile
from concourse.alu_op_type import AluOpType as OP

AF = mybir.ActivationFunctionType
F32 = mybir.dt.float32
F32R = mybir.dt.float32r
BF = mybir.dt.bfloat16
ROP = bass.bass_isa.ReduceOp

NCORES = 8
TOK = 2048
D = 1024
DI = 2048
HC = 4
CH = HC * 64          # 256
L = 256
NSTATE = 128
MYTOK = TOK // NCORES  # 256
WCOLS = 2 * CH + 2 * NSTATE + HC  # 772
EPS = 1e-5
TAU2 = 0.3
KRANK = 50


def build(upto=9, debug=False):
    nc = bacc.Bacc("TRN2", target_bir_lowering=False, debug=False, num_devices=NCORES)

    # ---------------- DRAM I/O ----------------
    x_bf = nc.dram_tensor("x_bf", [D, TOK], BF, kind="ExternalInput").ap()
    w_in = nc.dram_tensor("w_in", [D, WCOLS], BF, kind="ExternalInput").ap()
    conv_w = nc.dram_tensor("conv_w", [512, 4], F32, kind="ExternalInput").ap()
    conv_b = nc.dram_tensor("conv_b", [512], F32, kind="ExternalInput").ap()
    dt_bias = nc.dram_tensor("dt_bias", [HC, 1], F32, kind="ExternalInput").ap()
    alog32 = nc.dram_tensor("alog32", [32, 1], F32, kind="ExternalInput").ap()
    d_in = nc.dram_tensor("d_in", [128, 2], F32, kind="ExternalInput").ap()
    w_out = nc.dram_tensor("w_out", [DI, D], BF, kind="ExternalInput").ap()
    w_sc1 = nc.dram_tensor("w_sc1", [D, 256], BF, kind="ExternalInput").ap()
    w_sc2 = nc.dram_tensor("w_sc2", [256, 1], BF, kind="ExternalInput").ap()
    w_summ = nc.dram_tensor("w_summ", [D, 64], BF, kind="ExternalInput").ap()
    w_q = nc.dram_tensor("w_q", [D, 64], BF, kind="ExternalInput").ap()
    w_k = nc.dram_tensor("w_k", [64, 64], BF, kind="ExternalInput").ap()
    w_v = nc.dram_tensor("w_v", [64, 1024], BF, kind="ExternalInput").ap()
    w_gate = nc.dram_tensor("w_gate", [DI, D], BF, kind="ExternalInput").ap()
    x_myT = nc.dram_tensor("x_myT", [D, MYTOK], F32, kind="ExternalInput").ap()
    tok_ids = nc.dram_tensor("tok_ids", [MYTOK, 1], F32, kind="ExternalInput").ap()
    out_my = nc.dram_tensor("out_my", [D, MYTOK], F32, kind="ExternalOutput").ap()

    dbg = {}

    def dbg_out(name, shape, dt=F32, cond=True):
        if debug and cond:
            dbg[name] = nc.dram_tensor(name, shape, dt, kind="ExternalOutput").ap()
            return dbg[name]
        return None

    with tile.TileContext(nc) as tc, contextlib.ExitStack() as ES, \
            nc.allow_low_precision(reason="bf16 kernel validated vs fp32 ref"):
        _body(nc, tc, ES, upto, dbg_out, dict(
            x_bf=x_bf, w_in=w_in, conv_w=conv_w, conv_b=conv_b,
            dt_bias=dt_bias, alog32=alog32, d_in=d_in,
            w_out=w_out, w_sc1=w_sc1, w_sc2=w_sc2, w_summ=w_summ, w_q=w_q,
            w_k=w_k, w_v=w_v, w_gate=w_gate, x_myT=x_myT, tok_ids=tok_ids,
            out_my=out_my))
    nc.compile()
    return nc, dbg


def _body(nc, tc, ES, upto, dbg_out, io):
    alog32 = io["alog32"]
    (x_bf, w_in, conv_w, conv_b, dt_bias, d_in,
     w_out, w_sc1, w_sc2, w_summ, w_q, w_k, w_v, w_gate, x_myT, tok_ids, out_my) = (
        io["x_bf"], io["w_in"], io["conv_w"], io["conv_b"],
        io["dt_bias"], io["d_in"], io["w_out"],
        io["w_sc1"], io["w_sc2"], io["w_summ"], io["w_q"], io["w_k"], io["w_v"],
        io["w_gate"], io["x_myT"], io["tok_ids"], io["out_my"])
    pers = ES.enter_context(tc.tile_pool(name="pers", bufs=1))
    dram = ES.enter_context(tc.tile_pool(name="dram", bufs=1, space="DRAM"))
    # tail-weight tiles reserved up front (stack order); DMAs issued at stage 2
    wts = ES.enter_context(tc.tile_pool(name="wts", bufs=1))
    wo_all = wts.tile([128, 16, 1024], BF, tag="wo_all")
    wg_all = wts.tile([128, 16, 1024], BF, tag="wg_all")
    sc1_all = wts.tile([128, 8, 256], BF, tag="sc1_all")
    wsm_all = wts.tile([128, 8, 64], BF, tag="wsm_all")
    wq_all = wts.tile([128, 8, 64], BF, tag="wq_all")
    wv_all = wts.tile([64, 8, 128], BF, tag="wv_all")
    wkt = wts.tile([64, 64], BF, tag="wkt")
    w2c = wts.tile([128, 2], BF, tag="w2c")

    # ---- shared constants ----
    ident = pers.tile([128, 128], F32, tag="ident")
    with tc.tile_pool(name="tcst", bufs=1) as tcst:
        iod = tcst.tile([128, 128], F32, tag="iod")
        nc.gpsimd.iota(iod[:], pattern=[[1, 128]], base=0, channel_multiplier=-1,
                       allow_small_or_imprecise_dtypes=True)
        nc.vector.tensor_scalar(out=ident[:], in0=iod[:], scalar1=0.0, scalar2=None,
                                op0=OP.is_equal)
    identb = pers.tile([128, 128], BF, tag="identb")
    nc.vector.tensor_copy(identb[:], ident[:])
    identr = pers.tile([128, 128], F32R, tag="identr")
    nc.vector.tensor_copy(identr[:], ident[:])
    epsc = pers.tile([128, 1], F32, tag="epsc")
    nc.vector.memset(epsc[:], EPS)
    ones32 = pers.tile([32, 256], BF, tag="ones32")
    nc.vector.memset(ones32[:], 1.0)
    onescolb = pers.tile([128, 1], BF, tag="onescolb")
    nc.vector.memset(onescolb[:], 1.0)
    onesf = pers.tile([128, 128], F32, tag="onesf")
    nc.vector.memset(onesf[:], 1.0)
    onescol = pers.tile([128, 1], F32R, tag="onescol")
    nc.vector.tensor_copy(onescol[:], onesf[:, 0:1])
    onesb = pers.tile([128, 128], F32R, tag="onesb")
    nc.vector.tensor_copy(onesb[:], onesf[:])

    # ---------------- stage 0+1: rmsnorm stats + in_proj ----------------
    mid_stack = contextlib.ExitStack()
    mid = mid_stack.enter_context(tc.tile_pool(name="mid", bufs=1))
    cv_stack = contextlib.ExitStack()
    cvp = cv_stack.enter_context(tc.tile_pool(name="cvp", bufs=1))
    zT = mid.tile([128, 2, TOK], BF, tag="zT")
    cvin = cvp.tile([128, 4, 2, 1028], BF, tag="cvin")
    dtraw = mid.tile([HC, TOK], F32, tag="dtraw")
    nc.vector.memset(cvin[:, :, :, 0:4], 0.0)

    with tc.tile_pool(name="big", bufs=1) as big, \
         tc.tile_pool(name="t01", bufs=2) as t01, \
         tc.tile_pool(name="psq", bufs=1, space="PSUM") as psq, \
         tc.tile_pool(name="psm", bufs=1, space="PSUM") as psm:
        xTb = big.tile([128, 8, TOK], BF, tag="xTb")
        xbv = x_bf.rearrange("(a p) t -> p a t", p=128)
        for a in range(8):
            nc.sync.dma_start(xTb[:, a, :], xbv[:, a, :])
        wz = big.tile([128, 8, 256], BF, tag="wz")
        wx = big.tile([128, 8, 256], BF, tag="wx")
        wbc = big.tile([128, 8, 256], BF, tag="wbc")
        wdt = big.tile([128, 8, HC], BF, tag="wdt")
        wiv = w_in.rearrange("(a p) c -> p a c", p=128)
        nc.sync.dma_start(wz[:], wiv[:, :, 0:256])
        nc.sync.dma_start(wx[:], wiv[:, :, 256:512])
        nc.sync.dma_start(wbc[:], wiv[:, :, 512:768])
        nc.sync.dma_start(wdt[:], wiv[:, :, 768:772])

        # rstd per token: sum of squares via ones-matmul, then Rsqrt
        s_bc = big.tile([128, TOK], F32, tag="s_bc")
        srow = big.tile([1, TOK], F32, tag="srow")
        accs_q = [psq.tile([1, 512], F32, name=f"psq{n}", tag=f"psq{n}")
                  for n in range(4)]
        for a in range(8):
            sq = t01.tile([128, TOK], F32R, tag="sq")
            nc.scalar.activation(sq[:], xTb[:, a, :], AF.Square)
            for n in range(4):
                nc.tensor.matmul(accs_q[n][:], onescol[:], sq[:, 512 * n:512 * (n + 1)],
                                 start=(a == 0), stop=(a == 7))
        srt = big.tile([1, TOK], F32, tag="srt")
        for n in range(4):
            nc.scalar.activation(srt[0:1, 512 * n:512 * (n + 1)], accs_q[n][:],
                                 AF.Sqrt, bias=epsc[0:1, 0:1], scale=1.0 / D)
        # broadcast sqrt first, then wide reciprocal (single-partition
        # reciprocal on [1,2048] costs ~13us; [128,2048] costs ~2us)
        nc.gpsimd.partition_broadcast(s_bc[:], srt[:], channels=128)
        nc.vector.reciprocal_approx_fast(out=s_bc[:], in_=s_bc[:])

        for mi, m in enumerate((6, 4, 5, 2, 3, 0, 1)):
            mm_m = 4 if m == 6 else 128
            accs = [psm.tile([128, 512], F32, name=f"mmacc{n}", tag=f"mmacc{n}")
                    for n in range(4)]
            for k in range(8):
                if m == 6:
                    lhs = wdt[:, k, :]
                else:
                    w_t = (wz, wx, wbc)[m // 2]
                    coff = (m % 2) * 128
                    lhs = w_t[:, k, coff:coff + 128]
                for n in range(4):
                    n0 = 512 * n
                    nc.tensor.matmul(accs[n][0:mm_m, :], lhs, xTb[:, k, n0:n0 + 512],
                                     start=(k == 0), stop=(k == 7))
            for n in range(4):
                n0 = 512 * n
                sb = s_bc[0:mm_m, n0:n0 + 512]
                if m < 2:
                    dst = zT[:, m, n0:n0 + 512]
                elif m < 6:
                    b = n0 // 1024
                    dst = cvin[:, m - 2, b, 4 + (n0 % 1024):4 + (n0 % 1024) + 512]
                else:
                    dst = dtraw[:, n0:n0 + 512]
                nc.vector.tensor_tensor(dst, accs[n][0:mm_m, :], sb, OP.mult)

    zx_dbg = dbg_out("zx_dbg", [768, TOK], BF)
    if zx_dbg is not None:
        nc.sync.dma_start(zx_dbg[0:256, :].rearrange("(a p) t -> p a t", p=128), zT[:])
        for mi in range(4):
            for b in range(2):
                nc.sync.dma_start(
                    zx_dbg[256 + 128 * mi:384 + 128 * mi, 1024 * b:1024 * (b + 1)],
                    cvin[:, mi, b, 4:1028])
    dtr_dbg = dbg_out("dtr_dbg", [HC, TOK], F32)
    if dtr_dbg is not None:
        nc.sync.dma_start(dtr_dbg[:], dtraw[:])
    if upto < 2:
        cv_stack.close(); mid_stack.close()
        return

    # ---------------- stage 2: conv+silu, dt/dA/cs ----------------
    ssd_stack = contextlib.ExitStack()
    ssd = ssd_stack.enter_context(tc.tile_pool(name="ssd", bufs=1))
    xsT = ssd.tile([128, 2, TOK], BF, tag="xsT")
    bT = ssd.tile([128, TOK], BF, tag="bT")
    cT = ssd.tile([128, TOK], BF, tag="cT")
    dtt = ssd.tile([HC, TOK], F32, tag="dtt")
    yT = ssd.tile([128, 2, TOK], BF, tag="yT")
    g = ssd.tile([128, 2, TOK], BF, tag="g")

    cwc = pers.tile([128, 4, 4], F32, tag="cwc")
    cbc = pers.tile([128, 4], F32, tag="cbc")
    nc.sync.dma_start(cwc[:], conv_w.rearrange("(a p) k -> p a k", p=128))
    nc.sync.dma_start(cbc[:], conv_b.rearrange("(a p) -> p a", p=128))

    # depthwise conv as 4 shifted diagonal matmuls accumulating in PSUM
    with tc.tile_pool(name="cvt", bufs=2) as cvt, \
         tc.tile_pool(name="cps", bufs=2, space="PSUM") as cps:
        dwk = cvt.tile([128, 4, 4, 128], BF, tag="dwk")
        for t in range(4):
            for k in range(4):
                nc.vector.tensor_scalar(out=dwk[:, t, k, :], in0=identb[:],
                                        scalar1=cwc[:, t, k:k + 1], scalar2=None,
                                        op0=OP.mult)
        for t in range(4):
            for b in range(2):
                for hv in range(2):
                    acc = cps.tile([128, 512], F32, tag="cacc")
                    o = 512 * hv
                    for k in range(4):
                        nc.tensor.matmul(acc[:], dwk[:, t, k, :],
                                         cvin[:, t, b, o + k + 1:o + 513 + k],
                                         start=(k == 0), stop=(k == 3))
                    bsl = slice(1024 * b + o, 1024 * b + o + 512)
                    dst = (xsT[:, t, bsl] if t < 2 else
                           (bT[:, bsl] if t == 2 else cT[:, bsl]))
                    nc.scalar.activation(dst, acc[:], AF.Silu, bias=cbc[:, t:t + 1])

    dtb_c = pers.tile([HC, 1], F32, tag="dtb_c")
    nc.sync.dma_start(dtb_c[:], dt_bias[:])
    alog32_c = pers.tile([32, 1], F32, tag="alog32_c")
    nc.sync.dma_start(alog32_c[:], alog32[:])
    # softplus(x+b) = -ln(sigmoid(-(x+b)))
    dtbn = pers.tile([HC, 1], F32, tag="dtbn")
    nc.vector.tensor_scalar(out=dtbn[:], in0=dtb_c[:], scalar1=-1.0, scalar2=None,
                            op0=OP.mult)
    nc.scalar.activation(dtt[:], dtraw[:], AF.Sigmoid, scale=-1.0, bias=dtbn[:, 0:1])
    nc.scalar.activation(dtt[:], dtt[:], AF.Ln)
    nc.vector.tensor_scalar(out=dtt[:], in0=dtt[:], scalar1=-1.0, scalar2=None,
                            op0=OP.mult)

    dt_dbg = dbg_out("dt_dbg", [HC, TOK], F32)
    if dt_dbg is not None:
        nc.sync.dma_start(dt_dbg[:], dtt[:])
    xbc_dbg = dbg_out("xbc_dbg", [512, TOK], BF)
    if xbc_dbg is not None:
        nc.sync.dma_start(xbc_dbg[0:256, :].rearrange("(a p) t -> p a t", p=128), xsT[:])
        nc.sync.dma_start(xbc_dbg[256:384, :], bT[:])
        nc.sync.dma_start(xbc_dbg[384:512, :], cT[:])
    if upto < 3:
        ssd_stack.close(); cv_stack.close(); mid_stack.close()
        return

    # ---------------- stage 3: SSD ----------------
    # dAr (32 rows = h*8 + b*4 + c, 256): DMA from dtt then scale by -exp(A_log)
    dAr = ssd.tile([32, L], F32, tag="dAr")
    csr = ssd.tile([32, L], F32R, tag="csr")
    expcs = ssd.tile([32, L], F32R, tag="expcs")
    expa32 = ssd.tile([32, 1], F32, tag="expa32")
    nc.scalar.activation(expa32[:], alog32_c[:], AF.Exp)
    for bc in range(8):
        b, c = bc // 4, bc % 4
        nc.sync.dma_start(dAr[bc:bc + 25:8, :],
                          dtt[:, 1024 * b + 256 * c:1024 * b + 256 * (c + 1)])
    nc.vector.tensor_scalar(out=dAr[:], in0=dAr[:], scalar1=expa32[:, 0:1], scalar2=-1.0,
                            op0=OP.mult, op1=OP.mult)
    nc.vector.tensor_tensor_scan(csr[:], dAr[:], dAr[:], 0.0, OP.add, OP.bypass)
    nc.scalar.activation(expcs[:], csr[:], AF.Exp)
    decay_r = ssd.tile([32, L], F32, tag="decay_r")
    nc.scalar.activation(decay_r[:], csr[:], AF.Exp, scale=-1.0, bias=csr[:, L - 1:L])

    # prefetch tail weights (overlaps SSD compute; behind the staging DMAs)
    nc.sync.dma_start(wo_all[:], w_out.rearrange("(a p) m -> p a m", p=128))
    nc.sync.dma_start(wg_all[:], w_gate.rearrange("(a p) m -> p a m", p=128))
    nc.sync.dma_start(sc1_all[:], w_sc1.rearrange("(a p) m -> p a m", p=128))
    nc.sync.dma_start(wsm_all[:], w_summ.rearrange("(a p) m -> p a m", p=128))
    nc.sync.dma_start(wq_all[:], w_q.rearrange("(a p) m -> p a m", p=128))
    nc.sync.dma_start(wv_all[:], w_v.rearrange("s (a m) -> s a m", m=128))
    nc.sync.dma_start(wkt[:], w_k[:])
    nc.sync.dma_start(w2c[:], w_sc2.rearrange("(a p) o -> p (a o)", p=128))

    # s-major columns: dec as (128, 2st, 32r) with r = h*8+bc; dt per (b,c)
    dec_col = ssd.tile([128, 2, 32], F32, tag="dec_col")
    dt_col = ssd.tile([128, 2, 8, HC], F32, tag="dt_col")
    dtdec_col = ssd.tile([128, 2, 8, HC], F32, tag="dtdec_col")
    cs_colT = ssd.tile([128, 2, 32], F32, tag="cs_colT")
    with tc.tile_pool(name="psmt", bufs=2, space="PSUM") as psmt:
        for st in range(2):
            pt = psmt.tile([128, 32], F32, tag="mt32")
            nc.tensor.transpose(pt[:], decay_r[:, 128 * st:128 * (st + 1)],
                                ident[0:32, 0:32])
            nc.scalar.activation(dec_col[:, st, :], pt[:], AF.Copy)
            ptc = psmt.tile([128, 32], F32, tag="mt32c")
            nc.tensor.transpose(ptc[:].bitcast(F32R), csr[:, 128 * st:128 * (st + 1)],
                                identr[0:32, 0:32])
            nc.scalar.activation(cs_colT[:, st, :], ptc[:], AF.Copy)
            for bc in range(8):
                b, c = bc // 4, bc % 4
                pt2 = psmt.tile([128, HC], F32, tag="mt")
                t0 = 1024 * b + 256 * c + 128 * st
                nc.tensor.transpose(pt2[:], dtt[:, t0:t0 + 128], ident[0:HC, 0:HC])
                nc.scalar.activation(dt_col[:, st, bc, :], pt2[:], AF.Copy)
            for bc in range(8):
                nc.vector.tensor_tensor(dtdec_col[:, st, bc, :], dt_col[:, st, bc, :],
                                        dec_col[:, st, bc:bc + 25:8], OP.mult)

    # exp(cs_last) per row, broadcast to all partitions: etb [128, 32]
    etb = ssd.tile([128, 32], F32, tag="etb")
    with tc.tile_pool(name="pset", bufs=1, space="PSUM") as pset, \
         tc.tile_pool(name="tet", bufs=1) as tet:
        ptl = pset.tile([1, 32], F32, tag="ptl")
        nc.tensor.transpose(ptl[:].bitcast(F32R), csr[:, L - 1:L], identr[0:32, 0:32])
        etrow = tet.tile([1, 32], F32, tag="etrow")
        nc.scalar.activation(etrow[:], ptl[:], AF.Exp)
        nc.gpsimd.partition_broadcast(etb[:], etrow[:], channels=128)

    # mask bias: mb[p, st, l] = 0 if l >= 128*st+p else -1e30
    maskb = ssd.tile([128, 2, L], F32, tag="maskb")
    with tc.tile_pool(name="tio2", bufs=2) as tio2:
        for st in range(2):
            iol2 = tio2.tile([128, L], F32, tag="iol2")
            nc.gpsimd.iota(iol2[:], pattern=[[1, L]], base=-128 * st,
                           channel_multiplier=-1, allow_small_or_imprecise_dtypes=True)
            nc.vector.tensor_scalar(out=maskb[:, st, :], in0=iol2[:], scalar1=0.0,
                                    scalar2=-1e30, op0=OP.is_lt, op1=OP.mult)

    # D per pair-of-heads column (pre-broadcast on host)
    dDc = pers.tile([128, 2], F32, tag="dDc")
    nc.sync.dma_start(dDc[:], d_in[:])


    a2a_in = dram.tile([NCORES, CH, MYTOK], BF, tag="a2a_in")
    a2a_out = dram.tile([NCORES, CH, MYTOK], BF, tag="a2a_out")

    Rr = ssd.tile([128, 2, CH], BF, tag="Rr")
    with tc.tile_pool(name="psT", bufs=1, space="PSUM") as psT, \
         tc.tile_pool(name="psS2", bufs=1, space="PSUM") as psS2, \
         tc.tile_pool(name="psG", bufs=1, space="PSUM") as psG, \
         tc.tile_pool(name="psY", bufs=2, space="PSUM") as psY, \
         tc.tile_pool(name="psB", bufs=1, space="PSUM") as psB, \
         tc.tile_pool(name="tT", bufs=2) as tT, \
         tc.tile_pool(name="tG", bufs=3) as tG, \
         tc.tile_pool(name="tM", bufs=3) as tM:
        for b in range(2):
            for c in range(4):
                bc = b * 4 + c
                t0 = 1024 * b + 256 * c
                # csr/expcs rows for this chunk staged at partitions {0,64}
                # (PE base-partition alignment); scalar-queue DMAs so they
                # don't queue behind the big weight prefetch
                cs4a = tT.tile([128, L], F32R, tag="cs4a")
                cs4b = tT.tile([128, L], F32R, tag="cs4b")
                nc.scalar.dma_start(cs4a[0:65:64, :], csr[bc:bc + 9:8, :])
                nc.scalar.dma_start(cs4b[0:65:64, :], csr[bc + 16:bc + 25:8, :])
                if c > 0:
                    ec4a = tT.tile([128, L], F32R, tag="ec4a")
                    ec4b = tT.tile([128, L], F32R, tag="ec4b")
                    nc.scalar.dma_start(ec4a[0:65:64, :], expcs[bc:bc + 9:8, :])
                    nc.scalar.dma_start(ec4b[0:65:64, :], expcs[bc + 16:bc + 25:8, :])
                # per-chunk transposes: xd/xdd (s-major), bS
                xd = tT.tile([128, 2, CH], BF, tag="xd")
                xdd = tT.tile([128, 2, CH], BF, tag="xdd")
                bS = tT.tile([128, 2, NSTATE], BF, tag="bS")
                for st in range(2):
                    ts0 = t0 + 128 * st
                    for cb in range(2):
                        pt = psT.tile([128, 128], BF, tag="xdt")
                        nc.tensor.transpose(pt[:], xsT[:, cb, ts0:ts0 + 128], identb[:])
                        for hh in range(2):
                            h = 2 * cb + hh
                            nc.vector.tensor_scalar(
                                out=xd[:, st, 64 * h:64 * (h + 1)],
                                in0=pt[:, 64 * hh:64 * (hh + 1)],
                                scalar1=dt_col[:, st, bc, h:h + 1], scalar2=None,
                                op0=OP.mult)
                            nc.vector.tensor_scalar(
                                out=xdd[:, st, 64 * h:64 * (h + 1)],
                                in0=pt[:, 64 * hh:64 * (hh + 1)],
                                scalar1=dtdec_col[:, st, bc, h:h + 1], scalar2=None,
                                op0=OP.mult)
                    ptb = psT.tile([128, 128], BF, tag="bdt")
                    nc.tensor.transpose(ptb[:], bT[:, ts0:ts0 + 128], identb[:])
                    nc.scalar.activation(bS[:, st, :], ptb[:], AF.Copy)
                # states S (n, (h,p))
                sps = psS2.tile([128, CH], F32, tag="sps")
                for st in range(2):
                    nc.tensor.matmul(sps[:], bS[:, st, :], xdd[:, st, :],
                                     start=(st == 0), stop=(st == 1))
                S = tT.tile([128, CH], F32, tag="S")
                nc.scalar.activation(S[:], sps[:], AF.Copy)
                # Gt (s,l) shared across heads
                gts = []
                for st in range(2):
                    pg = psG.tile([128, L], F32, tag="pg")
                    nc.tensor.matmul(pg[:], bT[:, t0 + 128 * st:t0 + 128 * (st + 1)],
                                     cT[:, t0:t0 + 256], start=True, stop=True)
                    gt = tG.tile([128, L], BF, tag="gt")
                    nc.scalar.activation(gt[:], pg[:], AF.Copy)
                    gts.append(gt)
                for h in range(HC):
                    cb, hh = h // 2, h % 2
                    r = h * 8 + bc
                    hp = 64 * (h % 2)
                    cs4 = cs4a if h < 2 else cs4b
                    # broadcast csr row r to all partitions via rank-1 matmul
                    csp = psB.tile([128, L], F32, tag="csp")
                    nc.tensor.matmul(csp[:], onesb[hp:hp + 1, :], cs4[hp:hp + 1, :],
                                     start=True, stop=True)
                    ce = None
                    if c > 0:
                        ec4 = ec4a if h < 2 else ec4b
                        ecp = psB.tile([128, L], F32, tag="ecp")
                        nc.tensor.matmul(ecp[:], onesb[hp:hp + 1, :], ec4[hp:hp + 1, :],
                                         start=True, stop=True)
                        ce = tM.tile([128, L], BF, tag="ce")
                        nc.vector.tensor_tensor(ce[:], cT[:, t0:t0 + 256], ecp[:],
                                                OP.mult)
                    psl = slice(64 * hh, 64 * (hh + 1))
                    ypb = psY.tile([64, L], F32, tag="ypb")
                    ltp2 = tM.tile([128, 2, L], F32, tag="ltp2")
                    for st in range(2):
                        nc.vector.scalar_tensor_tensor(
                            out=ltp2[:, st, :], in0=csp[:],
                            scalar=cs_colT[:, st, r:r + 1],
                            in1=maskb[:, st, :], op0=OP.subtract, op1=OP.add)
                    lt2 = tM.tile([128, 2, L], BF, tag="lt2")
                    nc.scalar.activation(lt2[:], ltp2[:], AF.Exp)
                    for st in range(2):
                        ms = tM.tile([128, L], BF, tag="ms")
                        nc.gpsimd.tensor_tensor(ms[:], lt2[:, st, :], gts[st][:],
                                                OP.mult)
                        nc.tensor.matmul(ypb[:], xd[:, st, 64 * h:64 * (h + 1)],
                                         ms[:], start=(st == 0),
                                         stop=(st == 1 and c == 0))
                    if c > 0:
                        nc.tensor.matmul(ypb[:], Rr[:, b, 64 * h:64 * (h + 1)],
                                         ce[:], start=False, stop=True)
                    nc.vector.scalar_tensor_tensor(
                        out=yT[psl, cb, t0:t0 + 256], in0=xsT[psl, cb, t0:t0 + 256],
                        scalar=dDc[psl, cb:cb + 1], in1=ypb[0:64, :],
                        op0=OP.mult, op1=OP.add)
                # R update for next chunk
                if c == 0:
                    nc.scalar.activation(Rr[:, b, :], S[:], AF.Copy)
                elif c < 3:
                    for h in range(HC):
                        r = h * 8 + bc
                        nc.vector.scalar_tensor_tensor(
                            out=Rr[:, b, 64 * h:64 * (h + 1)],
                            in0=Rr[:, b, 64 * h:64 * (h + 1)],
                            scalar=etb[:, r:r + 1],
                            in1=S[:, 64 * h:64 * (h + 1)], op0=OP.mult, op1=OP.add)
                # gated z*silu for this chunk + A2A staging (dest core j == bc)
                for cb in range(2):
                    sz = tM.tile([128, 256], BF, tag="sz")
                    nc.scalar.activation(sz[:], zT[:, cb, t0:t0 + 256], AF.Silu)
                    nc.gpsimd.tensor_tensor(g[:, cb, t0:t0 + 256],
                                            yT[:, cb, t0:t0 + 256], sz[:], OP.mult)
                    nc.sync.dma_start(a2a_in[bc, 128 * cb:128 * (cb + 1), :],
                                      g[:, cb, t0:t0 + 256])

    y_dbg = dbg_out("y_dbg", [CH, TOK], BF)
    if y_dbg is not None:
        nc.sync.dma_start(y_dbg.rearrange("(a p) t -> p a t", p=128), yT[:])
    g_dbg = dbg_out("g_dbg", [CH, TOK], BF)
    if g_dbg is not None:
        nc.sync.dma_start(g_dbg.rearrange("(a p) t -> p a t", p=128), g[:])
    if upto < 5:
        ssd_stack.close(); cv_stack.close(); mid_stack.close()
        return

    # ---------------- stage 5: AllToAll ----------------
    ssd_stack.close()
    cv_stack.close()
    mid_stack.close()
    nc.gpsimd.collective_compute(
        "AllToAll", mybir.AluOpType.bypass,
        replica_groups=[list(range(NCORES))],
        ins=[a2a_in.opt()], outs=[a2a_out.opt()],
    )
    st6 = ES.enter_context(tc.tile_pool(name="st6", bufs=1))
    g2 = st6.tile([128, 16, MYTOK], BF, tag="g2")
    nc.sync.dma_start(g2[:], a2a_out.rearrange("j (cb p) t -> p (j cb) t", p=128))

    # ---------------- stage 5b: gated RMSNorm (token-local) ----------------
    # ssm_norm_w is folded into w_out on the host, so yn = g2 * rstd only.
    yn = st6.tile([128, 16, MYTOK], BF, tag="yn")
    rstd_bc = st6.tile([128, MYTOK], F32, tag="rstd_bc")
    with tc.tile_pool(name="tn", bufs=3) as tn, \
         tc.tile_pool(name="psn", bufs=1, space="PSUM") as psn:
        ssps = psn.tile([1, MYTOK], F32, tag="ssps")
        for i in range(16):
            gsq = tn.tile([128, MYTOK], BF, tag="gsq")
            nc.scalar.activation(gsq[:], g2[:, i, :], AF.Square)
            nc.tensor.matmul(ssps[:], onescolb[:], gsq[:], start=(i == 0),
                             stop=(i == 15))
        rstd_s = tn.tile([1, MYTOK], F32, tag="rstd_s")
        nc.scalar.activation(rstd_s[:], ssps[:], AF.Sqrt, bias=epsc[0:1, 0:1],
                             scale=1.0 / DI)
        nc.gpsimd.partition_broadcast(rstd_bc[:], rstd_s[:], channels=128)
        nc.vector.reciprocal_approx_fast(out=rstd_bc[:], in_=rstd_bc[:])
    for i in range(16):
        nc.vector.tensor_tensor(yn[:, i, :], g2[:, i, :], rstd_bc[:], OP.mult)
    yn_dbg = dbg_out("yn_dbg", [DI, MYTOK], BF)
    if yn_dbg is not None:
        nc.sync.dma_start(yn_dbg.rearrange("(a p) t -> p a t", p=128), yn[:])
    if upto < 6:
        return

    # ---------------- stage 6a: out_proj, scorer, summaries, q ----------------
    y2 = st6.tile([128, 8, MYTOK], BF, tag="y2")
    with tc.tile_pool(name="ps6", bufs=2, space="PSUM") as ps6:
        for m in range(8):
            acc = ps6.tile([128, MYTOK], F32, tag="oacc")
            for k in range(16):
                nc.tensor.matmul(acc[:], wo_all[:, k, 128 * m:128 * (m + 1)], yn[:, k, :],
                                 start=(k == 0), stop=(k == 15))
            if m % 2 == 0:
                nc.vector.tensor_copy(y2[:, m, :], acc[:])
            else:
                nc.scalar.activation(y2[:, m, :], acc[:], AF.Copy)

        # scorer
        rl1 = st6.tile([128, 2, MYTOK], BF, tag="rl1")
        for m in range(2):
            acc = ps6.tile([128, MYTOK], F32, tag="oacc")
            for k in range(8):
                nc.tensor.matmul(acc[:], sc1_all[:, k, 128 * m:128 * (m + 1)], y2[:, k, :],
                                 start=(k == 0), stop=(k == 7))
            nc.scalar.activation(rl1[:, m, :], acc[:], AF.Relu)
        u_row = st6.tile([1, MYTOK], F32, tag="u_row")
        ups = ps6.tile([1, MYTOK], F32, tag="ups")
        for m in range(2):
            nc.tensor.matmul(ups[:], w2c[:, m:m + 1], rl1[:, m, :], start=(m == 0),
                             stop=(m == 1))
        nc.vector.tensor_copy(u_row[:], ups[:])

        # summaries + q (64-col projections of y2)
        summT = st6.tile([64, MYTOK], BF, tag="summT")
        qT = st6.tile([64, MYTOK], BF, tag="qT")
        for (wv_t, dst) in ((wsm_all, summT), (wq_all, qT)):
            acc = ps6.tile([64, MYTOK], F32, tag="sacc6")
            for k in range(8):
                nc.tensor.matmul(acc[:], wv_t[:, k, :], y2[:, k, :], start=(k == 0),
                                 stop=(k == 7))
            nc.scalar.activation(dst[:], acc[:], AF.Copy)

        # summaries token-major (for allgather), as f32
        stm = st6.tile([128, 2, 64], F32, tag="stm")
        for st in range(2):
            pt = ps6.tile([128, 64], BF, tag="stp")
            nc.tensor.transpose(pt[:], summT[:, 128 * st:128 * (st + 1)],
                                identb[0:64, 0:64])
            nc.scalar.activation(stm[:, st, :], pt[:], AF.Copy)

    # gate phase 1: y2 half (overlaps the pool-selection collectives)
    gy2 = st6.tile([128, 8, MYTOK], F32, tag="gy2")
    with tc.tile_pool(name="psg1", bufs=2, space="PSUM") as psg1:
        for m in range(8):
            acc = psg1.tile([128, MYTOK], F32, tag="g1acc")
            for k in range(8):
                nc.tensor.matmul(acc[:], wg_all[:, k, 128 * m:128 * (m + 1)], y2[:, k, :],
                                 start=(k == 0), stop=(k == 7))
            nc.scalar.activation(gy2[:, m, :], acc[:], AF.Copy)

    y2_dbg = dbg_out("y2_dbg", [D, MYTOK], BF)
    if y2_dbg is not None:
        nc.sync.dma_start(y2_dbg.rearrange("(a p) t -> p a t", p=128), y2[:])
    u_dbg = dbg_out("u_dbg", [1, MYTOK], F32)
    if u_dbg is not None:
        nc.sync.dma_start(u_dbg[:], u_row[:])
    if upto < 7:
        return

    # ---------------- stage 6b: allgather u+summaries, ranks, members ----------------
    # member-phase constants hoisted off the post-AG1 critical chain
    iota_all = st6.tile([128, TOK], F32, tag="iota_all")
    nc.gpsimd.iota(iota_all[:], pattern=[[1, TOK]], base=0, channel_multiplier=0,
                   allow_small_or_imprecise_dtypes=True)
    tid_col = st6.tile([128, 2], F32, tag="tid_col")
    nc.sync.dma_start(tid_col[:], tok_ids.rearrange("(a p) o -> p (a o)", p=128))
    u_col = st6.tile([128, 2], F32, tag="u_col")
    with tc.tile_pool(name="psu", bufs=2, space="PSUM") as psu:
        for st in range(2):
            pu = psu.tile([128, 1], F32, tag="pu")
            nc.tensor.transpose(pu[:], u_row[0:1, 128 * st:128 * (st + 1)],
                                ident[0:1, 0:1])
            nc.vector.tensor_copy(u_col[:, st:st + 1], pu[:])

    ag1_in = dram.tile([MYTOK, 65], F32, tag="ag1_in")
    ag1_out = dram.tile([NCORES, MYTOK, 65], F32, tag="ag1_out")
    nc.sync.dma_start(ag1_in[:, 0:1].rearrange("t o -> o t"), u_row[:])
    for st in range(2):
        nc.sync.dma_start(ag1_in[128 * st:128 * (st + 1), 1:65], stm[:, st, :])
    nc.gpsimd.collective_compute(
        "AllGather", mybir.AluOpType.bypass,
        replica_groups=[list(range(NCORES))],
        ins=[ag1_in.opt()], outs=[ag1_out.opt()],
    )
    u_all = st6.tile([1, TOK], F32, tag="u_all")
    nc.sync.dma_start(u_all[:], ag1_out[:, :, 0:1].rearrange("j t o -> o (j t)"))
    summ_all = st6.tile([128, 16, 64], F32, tag="summ_all")
    summ_allr = st6.tile([128, 16, 64], BF, tag="summ_allr")
    nc.sync.dma_start(summ_all[:],
                      ag1_out[:, :, 1:65].rearrange("j (st p) c -> p (j st) c", p=128))

    # ranks for my 256 tokens vs all 2048; broadcast u via PE rank-1 matmul
    # (keeps the critical chain off the busy gpsimd queue)
    u_bc = st6.tile([128, TOK], F32, tag="u_bc")
    with tc.tile_pool(name="pub", bufs=2, space="PSUM") as pub:
        for n in range(4):
            pb = pub.tile([128, 512], F32, tag="pb")
            nc.tensor.matmul(pb[:], onesb[0:1, :],
                             u_all[0:1, 512 * n:512 * (n + 1)].bitcast(F32R),
                             start=True, stop=True)
            nc.scalar.activation(u_bc[:, 512 * n:512 * (n + 1)], pb[:], AF.Copy)
    member = st6.tile([128, 2], F32, tag="member")
    with tc.tile_pool(name="trk", bufs=1) as trk:
        for st in range(2):
            junk = trk.tile([128, TOK], F32, tag="junk")
            rgt = trk.tile([128, 1], F32, tag="rgt")
            nc.vector.tensor_scalar(out=junk[:], in0=u_bc[:], scalar1=u_col[:, st:st + 1],
                                    scalar2=0.0, op0=OP.is_gt, op1=OP.add,
                                    accum_out=rgt[:])
            eqm = trk.tile([128, TOK], F32, tag="eqm")
            nc.vector.tensor_scalar(out=eqm[:], in0=u_bc[:], scalar1=u_col[:, st:st + 1],
                                    scalar2=None, op0=OP.is_equal)
            junk2 = trk.tile([128, TOK], F32, tag="junk2")
            req = trk.tile([128, 1], F32, tag="req")
            nc.vector.scalar_tensor_tensor(out=junk2[:], in0=iota_all[:],
                                           scalar=tid_col[:, st:st + 1], in1=eqm[:],
                                           op0=OP.is_lt, op1=OP.mult, accum_out=req[:])
            rank = trk.tile([128, 1], F32, tag="rank")
            nc.vector.tensor_tensor(rank[:], rgt[:], req[:], OP.add)
            rlt = trk.tile([128, 1], F32, tag="rlt")
            nc.vector.tensor_scalar(out=rlt[:], in0=rank[:], scalar1=float(KRANK),
                                    scalar2=None, op0=OP.is_lt)
            vld = trk.tile([128, 1], F32, tag="vld")
            nc.vector.tensor_scalar(out=vld[:], in0=u_col[:, st:st + 1], scalar1=0.0,
                                    scalar2=None, op0=OP.is_gt)
            nc.vector.tensor_tensor(member[:, st:st + 1], rlt[:], vld[:], OP.mult)

    ag2_in = dram.tile([MYTOK, 1], F32, tag="ag2_in")
    ag2_out = dram.tile([NCORES, MYTOK, 1], F32, tag="ag2_out")
    for st in range(2):
        nc.sync.dma_start(ag2_in[128 * st:128 * (st + 1), :], member[:, st:st + 1])
    nc.gpsimd.collective_compute(
        "AllGather", mybir.AluOpType.bypass,
        replica_groups=[list(range(NCORES))],
        ins=[ag2_in.opt()], outs=[ag2_out.opt()],
    )

    # ---------------- stage 6c: retrieval (logits/exp overlap AG2) ----------------
    summT_all = st6.tile([64, TOK], BF, tag="summT_all")
    esm = st6.tile([128, 16, MYTOK], BF, tag="esm")
    retrT = st6.tile([128, 8, MYTOK], BF, tag="retrT")
    # cond from u_all (overlaps AG2 as well)
    cond_col = st6.tile([128, 1], F32, tag="cond_col")
    with tc.tile_pool(name="tcd", bufs=1) as tcd:
        sgj = tcd.tile([1, TOK], F32, tag="sgj")
        sgs = tcd.tile([1, 1], F32, tag="sgs")
        nc.scalar.activation(sgj[:], u_all[:], AF.Sigmoid, accum_out=sgs[:])
        vj = tcd.tile([1, TOK], F32, tag="vj")
        vs = tcd.tile([1, 1], F32, tag="vs")
        nc.vector.tensor_scalar(out=vj[:], in0=u_all[:], scalar1=0.0, scalar2=0.0,
                                op0=OP.is_gt, op1=OP.add, accum_out=vs[:])
        c1 = tcd.tile([1, 1], F32, tag="c1")
        nc.vector.tensor_scalar(out=c1[:], in0=sgs[:], scalar1=float(TAU2 * TOK),
                                scalar2=None, op0=OP.is_gt)
        c2 = tcd.tile([1, 1], F32, tag="c2")
        nc.vector.tensor_scalar(out=c2[:], in0=vs[:], scalar1=0.0, scalar2=None,
                                op0=OP.is_gt)
        cnd = tcd.tile([1, 1], F32, tag="cnd")
        nc.vector.tensor_tensor(cnd[:], c1[:], c2[:], OP.mult)
        nc.gpsimd.partition_broadcast(cond_col[:], cnd[:], channels=128)

    xres = st6.tile([128, 8, MYTOK], F32, tag="xres")
    nc.sync.dma_start(xres[:], x_myT.rearrange("(a p) t -> p a t", p=128))

    with tc.tile_pool(name="tr6", bufs=4) as tr6, \
         tc.tile_pool(name="psr6", bufs=1, space="PSUM") as psr6, \
         tc.tile_pool(name="psl6", bufs=2, space="PSUM") as psl6, \
         tc.tile_pool(name="psrp", bufs=1, space="PSUM") as psrp, \
         tc.tile_pool(name="psq6", bufs=1, space="PSUM") as psq6:
        for i in range(16):
            pt = psq6.tile([64, 128], F32, tag="satp")
            nc.tensor.transpose(pt[:], summ_all[:, i, :], ident[:])
            nc.scalar.activation(summT_all[:, 128 * i:128 * (i + 1)], pt[:], AF.Copy)
        wktp = psr6.tile([64, 64], BF, tag="wktp")
        nc.tensor.transpose(wktp[:], wkt[:], identb[0:64, 0:64])
        kwT = tr6.tile([64, 64], BF, tag="kwT")
        nc.scalar.activation(kwT[:], wktp[:], AF.Copy)
        kqp = psr6.tile([64, MYTOK], F32, tag="kqp")
        nc.tensor.matmul(kqp[:], kwT[:], qT[:], start=True, stop=True)
        kq = tr6.tile([64, MYTOK], BF, tag="kq")
        nc.vector.tensor_scalar(out=kq[:], in0=kqp[:], scalar1=0.25, scalar2=None,
                                op0=OP.mult)
        # unmasked exp(logits) — mask applied multiplicatively after AG2
        for i in range(16):
            lp = psl6.tile([128, MYTOK], F32, tag="lp")
            nc.tensor.matmul(lp[:], summT_all[:, 128 * i:128 * (i + 1)], kq[:],
                             start=True, stop=True)
            nc.scalar.activation(esm[:, i, :], lp[:], AF.Exp)
        mask_col = st6.tile([128, 16], F32, tag="mask_col")
        nc.sync.dma_start(mask_col[:], ag2_out.rearrange("j (a p) o -> p (j a o)", p=128))
        mem_dbg = dbg_out("mem_dbg", [1, TOK], F32)
        if mem_dbg is not None:
            nc.sync.dma_start(mem_dbg[:], ag2_out.rearrange("j t o -> o (j t)"))
        # masked bf16 summaries (mask folded into the f32->bf16 cast) and
        # softmax denominator as a mask-vector matmul on the tensor engine
        maskb16 = tr6.tile([128, 16], BF, tag="maskb16")
        nc.vector.tensor_scalar(out=maskb16[:], in0=mask_col[:], scalar1=1.0,
                                scalar2=None, op0=OP.mult)
        for i in range(16):
            nc.scalar.activation(summ_allr[:, i, :], summ_all[:, i, :], AF.Copy,
                                 scale=mask_col[:, i:i + 1])
        denps = psr6.tile([1, MYTOK], F32, tag="denps")
        for i in range(16):
            nc.tensor.matmul(denps[:], maskb16[:, i:i + 1], esm[:, i, :],
                             start=(i == 0), stop=(i == 15))
        den_row = tr6.tile([1, MYTOK], F32, tag="den_row")
        nc.scalar.activation(den_row[:], denps[:], AF.Copy)
        rden_bc = tr6.tile([64, MYTOK], F32, tag="rden_bc")
        nc.gpsimd.partition_broadcast(rden_bc[:], den_row[:], channels=64)
        nc.vector.reciprocal_approx_fast(out=rden_bc[:], in_=rden_bc[:])
        tmpp = psr6.tile([64, MYTOK], F32, tag="tmpp")
        for i in range(16):
            nc.tensor.matmul(tmpp[:], summ_allr[:, i, :], esm[:, i, :], start=(i == 0),
                             stop=(i == 15))
        tmps = tr6.tile([64, MYTOK], BF, tag="tmps")
        nc.vector.tensor_tensor(tmps[:], tmpp[:], rden_bc[:], OP.mult)
        for m in range(8):
            rp = psrp.tile([128, MYTOK], F32, tag="rp")
            nc.tensor.matmul(rp[:], wv_all[:, m, :], tmps[:], start=True, stop=True)
            if m % 2 == 0:
                nc.vector.tensor_copy(retrT[:, m, :], rp[:])
            else:
                nc.scalar.activation(retrT[:, m, :], rp[:], AF.Copy)

    retr_dbg = dbg_out("retr_dbg", [D, MYTOK], BF)
    if retr_dbg is not None:
        nc.sync.dma_start(retr_dbg.rearrange("(a p) t -> p a t", p=128), retrT[:])
    if upto < 9:
        return

    # ---------------- stage 6d: gate, final ----------------
    with tc.tile_pool(name="psg6", bufs=3, space="PSUM") as psg6, \
         tc.tile_pool(name="tf6", bufs=3) as tf6:
        for m in range(8):
            acc = psg6.tile([128, MYTOK], F32, tag="gacc")
            for k in range(8, 16):
                nc.tensor.matmul(acc[:], wg_all[:, k, 128 * m:128 * (m + 1)],
                                 retrT[:, k - 8, :], start=(k == 8), stop=(k == 15))
            gl = tf6.tile([128, MYTOK], F32, tag="gl")
            nc.vector.tensor_tensor(gl[:], acc[:], gy2[:, m, :], OP.add)
            gsb = tf6.tile([128, MYTOK], F32, tag="gsb")
            nc.scalar.activation(gsb[:], gl[:], AF.Sigmoid)
            t1 = tf6.tile([128, MYTOK], F32, tag="t1")
            nc.vector.tensor_tensor(t1[:], gsb[:], retrT[:, m, :], OP.mult)
            t2 = tf6.tile([128, MYTOK], F32, tag="t2")
            nc.vector.scalar_tensor_tensor(out=t2[:], in0=t1[:], scalar=cond_col[:, 0:1],
                                           in1=y2[:, m, :], op0=OP.mult, op1=OP.add)
            fin = tf6.tile([128, MYTOK], F32, tag="fin")
            nc.vector.tensor_tensor(fin[:], t2[:], xres[:, m, :], OP.add)
            nc.sync.dma_start(out_my[128 * m:128 * (m + 1), :], fin[:])


# ---- host-side sharding ----


def make_in_maps(inputs):
    import ml_dtypes
    BF_NP = ml_dtypes.bfloat16
    x = np.asarray(inputs['x'], np.float32)
    x_tok = np.ascontiguousarray(x.reshape(2048, 1024))
    x_bf = np.ascontiguousarray(x_tok.T.astype(BF_NP))
    ipw = (np.asarray(inputs['in_proj_w'], np.float32)
           * np.asarray(inputs['norm_w'], np.float32)[:, None])
    cw = np.asarray(inputs['conv_w'], np.float32)
    cb = np.asarray(inputs['conv_b'], np.float32)
    w_out = np.ascontiguousarray(
        (np.asarray(inputs['ssm_norm_w'], np.float32)[:, None]
         * np.asarray(inputs['out_proj_w'], np.float32)).astype(BF_NP))
    w_sc1 = np.ascontiguousarray(np.asarray(inputs['scorer_w1'], np.float32).astype(BF_NP))
    w_sc2 = np.ascontiguousarray(np.asarray(inputs['scorer_w2'], np.float32).astype(BF_NP))
    w_summ = np.ascontiguousarray(np.asarray(inputs['summ_w'], np.float32).astype(BF_NP))
    w_q = np.ascontiguousarray(np.asarray(inputs['q_w'], np.float32).astype(BF_NP))
    w_k = np.ascontiguousarray(np.asarray(inputs['k_w'], np.float32).astype(BF_NP))
    w_v = np.ascontiguousarray(np.asarray(inputs['v_w'], np.float32).astype(BF_NP))
    w_gate = np.ascontiguousarray(np.asarray(inputs['gate_w'], np.float32).astype(BF_NP))
    in_maps = []
    for k in range(8):
        zc = ipw[:, 256 * k:256 * (k + 1)]
        xc = ipw[:, 2048 + 256 * k:2048 + 256 * (k + 1)]
        bcc = ipw[:, 4096:4352]
        dtc = ipw[:, 4352 + 4 * k:4352 + 4 * (k + 1)]
        w_my = np.ascontiguousarray(
            np.concatenate([zc, xc, bcc, dtc], axis=1).astype(BF_NP))
        conv_rows = np.concatenate([cw[256 * k:256 * (k + 1)], cw[2048:2304]], axis=0)
        convb_rows = np.concatenate([cb[256 * k:256 * (k + 1)], cb[2048:2304]], axis=0)
        m = {
            'x_bf': x_bf,
            'w_in': w_my,
            'conv_w': np.ascontiguousarray(conv_rows),
            'conv_b': np.ascontiguousarray(convb_rows),
            'dt_bias': np.ascontiguousarray(inputs['dt_bias'][4 * k:4 * (k + 1), None]).astype(np.float32),
            'alog32': np.ascontiguousarray(np.repeat(inputs['A_log'][4 * k:4 * (k + 1)], 8)[:, None]).astype(np.float32),
            'd_in': np.ascontiguousarray(
                np.stack([np.repeat(inputs['D'][4 * k:4 * k + 2], 64),
                          np.repeat(inputs['D'][4 * k + 2:4 * k + 4], 64)], axis=1)).astype(np.float32),
            'w_out': w_out,
            'w_sc1': w_sc1,
            'w_sc2': w_sc2,
            'w_summ': w_summ,
            'w_q': w_q,
            'w_k': w_k,
            'w_v': w_v,
            'w_gate': w_gate,
            'x_myT': np.ascontiguousarray(x_tok[256 * k:256 * (k + 1), :].T),
            'tok_ids': np.arange(256 * k, 256 * (k + 1), dtype=np.float32)[:, None],
        }
        in_maps.append(m)
    return in_maps


def gather_out(results):
    out = np.empty((2048, 1024), np.float32)
    for k in range(8):
        out[256 * k:256 * (k + 1), :] = results[k]['out_my'].T
    return out.reshape(2, 1024, 1024)


_CACHED = {}


def _get_nc():
    if "nc" not in _CACHED:
        _CACHED["nc"] = build(upto=9, debug=False)[0]
    return _CACHED["nc"]


def kernel(**inputs):
    from concourse import bass_utils
    nc = _get_nc()
    in_maps = make_in_maps(inputs)
    res = bass_utils.run_bass_kernel_spmd(nc, in_maps, core_ids=list(range(NCORES)))
    return gather_out(res.results)


# revision 69
# speedup vs baseline: 1.0279x; 1.0279x over previous
"""MemMambaBlock Trainium2 kernel (self-contained).

8-core SPMD: head-sharded in_proj/conv/SSD -> AllToAll -> token-sharded
norm/out_proj/scorer/top-50-pool/retrieval/gate. The sequential memory-pool
scan is replaced by an exact parallel top-50 selection (streaming top-k
equivalence; slot order is irrelevant because the masked softmax retrieval
is permutation-invariant over pool slots).

v1: bf16 weights/activations, prefetched tail weights, PE-broadcast decay
matrices (no vector scans), vector/gpsimd split, per-chunk A2A staging.
"""
import contextlib
import numpy as np
import concourse.bass as bass
import concourse.bacc as bacc
import concourse.mybir as mybir
import concourse.tile as tile
from concourse.alu_op_type import AluOpType as OP

AF = mybir.ActivationFunctionType
F32 = mybir.dt.float32
F32R = mybir.dt.float32r
BF = mybir.dt.bfloat16
ROP = bass.bass_isa.ReduceOp

NCORES = 8
TOK = 2048
D = 1024
DI = 2048
HC = 4
CH = HC * 64          # 256
L = 256
NSTATE = 128
MYTOK = TOK // NCORES  # 256
WCOLS = 2 * CH + 2 * NSTATE + HC  # 772
EPS = 1e-5
TAU2 = 0.3
KRANK = 50


def build(upto=9, debug=False):
    nc = bacc.Bacc("TRN2", target_bir_lowering=False, debug=False, num_devices=NCORES)

    # ---------------- DRAM I/O ----------------
    x_bf = nc.dram_tensor("x_bf", [D, TOK], BF, kind="ExternalInput").ap()
    w_in = nc.dram_tensor("w_in", [D, WCOLS], BF, kind="ExternalInput").ap()
    conv_w = nc.dram_tensor("conv_w", [512, 4], F32, kind="ExternalInput").ap()
    conv_b = nc.dram_tensor("conv_b", [512], F32, kind="ExternalInput").ap()
    dt_bias = nc.dram_tensor("dt_bias", [HC, 1], F32, kind="ExternalInput").ap()
    alog32 = nc.dram_tensor("alog32", [32, 1], F32, kind="ExternalInput").ap()
    d_in = nc.dram_tensor("d_in", [128, 2], F32, kind="ExternalInput").ap()
    w_out = nc.dram_tensor("w_out", [DI, D], BF, kind="ExternalInput").ap()
    w_sc1 = nc.dram_tensor("w_sc1", [D, 256], BF, kind="ExternalInput").ap()
    w_sc2 = nc.dram_tensor("w_sc2", [256, 1], BF, kind="ExternalInput").ap()
    w_summ = nc.dram_tensor("w_summ", [D, 64], BF, kind="ExternalInput").ap()
    w_q = nc.dram_tensor("w_q", [D, 64], BF, kind="ExternalInput").ap()
    w_k = nc.dram_tensor("w_k", [64, 64], BF, kind="ExternalInput").ap()
    w_v = nc.dram_tensor("w_v", [64, 1024], BF, kind="ExternalInput").ap()
    w_gate = nc.dram_tensor("w_gate", [DI, D], BF, kind="ExternalInput").ap()
    x_bc = nc.dram_tensor("x_bc", [D, 260], BF, kind="ExternalInput").ap()
    x_myT = nc.dram_tensor("x_myT", [D, MYTOK], F32, kind="ExternalInput").ap()
    tok_ids = nc.dram_tensor("tok_ids", [MYTOK, 1], F32, kind="ExternalInput").ap()
    out_my = nc.dram_tensor("out_my", [D, MYTOK], F32, kind="ExternalOutput").ap()

    dbg = {}

    def dbg_out(name, shape, dt=F32, cond=True):
        if debug and cond:
            dbg[name] = nc.dram_tensor(name, shape, dt, kind="ExternalOutput").ap()
            return dbg[name]
        return None

    with tile.TileContext(nc) as tc, contextlib.ExitStack() as ES, \
            nc.allow_low_precision(reason="bf16 kernel validated vs fp32 ref"):
        _body(nc, tc, ES, upto, dbg_out, dict(
            x_bf=x_bf, w_in=w_in, conv_w=conv_w, conv_b=conv_b,
            dt_bias=dt_bias, alog32=alog32, d_in=d_in,
            w_out=w_out, w_sc1=w_sc1, w_sc2=w_sc2, w_summ=w_summ, w_q=w_q,
            w_k=w_k, w_v=w_v, w_gate=w_gate, x_bc=x_bc, x_myT=x_myT,
            tok_ids=tok_ids, out_my=out_my))
    nc.compile()
    return nc, dbg


def _body(nc, tc, ES, upto, dbg_out, io):
    alog32 = io["alog32"]
    (x_bf, w_in, conv_w, conv_b, dt_bias, d_in,
     w_out, w_sc1, w_sc2, w_summ, w_q, w_k, w_v, w_gate, x_bc, x_myT,
     tok_ids, out_my) = (
        io["x_bf"], io["w_in"], io["conv_w"], io["conv_b"],
        io["dt_bias"], io["d_in"], io["w_out"],
        io["w_sc1"], io["w_sc2"], io["w_summ"], io["w_q"], io["w_k"], io["w_v"],
        io["w_gate"], io["x_bc"], io["x_myT"], io["tok_ids"], io["out_my"])
    pers = ES.enter_context(tc.tile_pool(name="pers", bufs=1))
    dram = ES.enter_context(tc.tile_pool(name="dram", bufs=1, space="DRAM"))
    # tail-weight tiles reserved up front (stack order); DMAs issued at stage 2
    wts = ES.enter_context(tc.tile_pool(name="wts", bufs=1))
    wo_all = wts.tile([128, 16, 1024], BF, tag="wo_all")
    wg_all = wts.tile([128, 16, 1024], BF, tag="wg_all")
    sc1_all = wts.tile([128, 8, 256], BF, tag="sc1_all")
    wsm_all = wts.tile([128, 8, 64], BF, tag="wsm_all")
    wq_all = wts.tile([128, 8, 64], BF, tag="wq_all")
    wv_all = wts.tile([64, 8, 128], BF, tag="wv_all")
    wkt = wts.tile([64, 64], BF, tag="wkt")
    w2c = wts.tile([128, 2], BF, tag="w2c")

    # ---- shared constants ----
    ident = pers.tile([128, 128], F32, tag="ident")
    with tc.tile_pool(name="tcst", bufs=1) as tcst:
        iod = tcst.tile([128, 128], F32, tag="iod")
        nc.gpsimd.iota(iod[:], pattern=[[1, 128]], base=0, channel_multiplier=-1,
                       allow_small_or_imprecise_dtypes=True)
        nc.vector.tensor_scalar(out=ident[:], in0=iod[:], scalar1=0.0, scalar2=None,
                                op0=OP.is_equal)
    identb = pers.tile([128, 128], BF, tag="identb")
    nc.vector.tensor_copy(identb[:], ident[:])
    identr = pers.tile([128, 128], F32R, tag="identr")
    nc.vector.tensor_copy(identr[:], ident[:])
    epsc = pers.tile([128, 1], F32, tag="epsc")
    nc.vector.memset(epsc[:], EPS)
    ones32 = pers.tile([32, 256], BF, tag="ones32")
    nc.vector.memset(ones32[:], 1.0)
    onescolb = pers.tile([128, 1], BF, tag="onescolb")
    nc.vector.memset(onescolb[:], 1.0)
    onesf = pers.tile([128, 128], F32, tag="onesf")
    nc.vector.memset(onesf[:], 1.0)
    onescol = pers.tile([128, 1], F32R, tag="onescol")
    nc.vector.tensor_copy(onescol[:], onesf[:, 0:1])
    onesb = pers.tile([128, 128], F32R, tag="onesb")
    nc.vector.tensor_copy(onesb[:], onesf[:])

    cwc = pers.tile([128, 4, 4], F32, tag="cwc")
    cbc = pers.tile([128, 4], F32, tag="cbc")
    nc.sync.dma_start(cwc[:], conv_w.rearrange("(a p) k -> p a k", p=128))
    nc.sync.dma_start(cbc[:], conv_b.rearrange("(a p) -> p a", p=128))
    dwk = pers.tile([128, 4, 4, 128], BF, tag="dwk")
    for t in range(4):
        for k in range(4):
            nc.vector.tensor_scalar(out=dwk[:, t, k, :], in0=identb[:],
                                    scalar1=cwc[:, t, k:k + 1], scalar2=None,
                                    op0=OP.mult)

    # ---------------- stage 0+1: rmsnorm stats + in_proj ----------------
    mid_stack = contextlib.ExitStack()
    mid = mid_stack.enter_context(tc.tile_pool(name="mid", bufs=1))
    cv_stack = contextlib.ExitStack()
    cvp = cv_stack.enter_context(tc.tile_pool(name="cvp", bufs=1))
    zT = mid.tile([128, 2, TOK], BF, tag="zT")
    cvin = cvp.tile([128, 2, 2, 1028], BF, tag="cvin")
    dtraw = mid.tile([HC, TOK], F32, tag="dtraw")
    nc.vector.memset(cvin[:, :, :, 0:4], 0.0)

    with tc.tile_pool(name="big", bufs=1) as big, \
         tc.tile_pool(name="t01", bufs=2) as t01, \
         tc.tile_pool(name="psq", bufs=1, space="PSUM") as psq, \
         tc.tile_pool(name="psm", bufs=1, space="PSUM") as psm:
        xTb = big.tile([128, 8, TOK], BF, tag="xTb")
        xbv = x_bf.rearrange("(a p) t -> p a t", p=128)
        for a in range(8):
            nc.sync.dma_start(xTb[:, a, :], xbv[:, a, :])
        wz = big.tile([128, 8, 256], BF, tag="wz")
        wx = big.tile([128, 8, 256], BF, tag="wx")
        wbc = big.tile([128, 8, 256], BF, tag="wbc")
        wdt = big.tile([128, 8, HC], BF, tag="wdt")
        wiv = w_in.rearrange("(a p) c -> p a c", p=128)
        nc.sync.dma_start(wz[:], wiv[:, :, 0:256])
        nc.sync.dma_start(wx[:], wiv[:, :, 256:512])
        nc.sync.dma_start(wbc[:], wiv[:, :, 512:768])
        nc.sync.dma_start(wdt[:], wiv[:, :, 768:772])

        # ---- B/C path, token-sharded: this core's 256 tokens (+4 halo) ----
        xbcb = big.tile([128, 8, 260], BF, tag="xbcb")
        nc.sync.dma_start(xbcb[:], x_bc.rearrange("(a p) t -> p a t", p=128))
        accs_q = [psq.tile([1, 512], F32, name=f"psq{n}", tag=f"psq{n}")
                  for n in range(4)]
        for a in range(8):
            sqb = t01.tile([128, 260], F32R, tag="sqb")
            nc.scalar.activation(sqb[:], xbcb[:, a, :], AF.Square)
            nc.tensor.matmul(accs_q[0][0:1, 0:260], onescol[:], sqb[:],
                             start=(a == 0), stop=(a == 7))
        srtw = big.tile([1, 260], F32, tag="srtw")
        nc.scalar.activation(srtw[:], accs_q[0][0:1, 0:260], AF.Sqrt,
                             bias=epsc[0:1, 0:1], scale=1.0 / D)
        s_bw = big.tile([128, 260], F32, tag="s_bw")
        nc.gpsimd.partition_broadcast(s_bw[:], srtw[:], channels=128)
        nc.vector.reciprocal_approx_fast(out=s_bw[:], in_=s_bw[:])
        cvbc = big.tile([128, 2, 260], BF, tag="cvbc")
        for mb in range(2):
            accb = psm.tile([128, 512], F32, name=f"accb{mb}", tag=f"mmacc{mb}")
            for k in range(8):
                nc.tensor.matmul(accb[:, 0:260], wbc[:, k, 128 * mb:128 * (mb + 1)],
                                 xbcb[:, k, :], start=(k == 0), stop=(k == 7))
            nc.vector.tensor_tensor(cvbc[:, mb, :], accb[:, 0:260], s_bw[:], OP.mult)
        bcm = big.tile([128, 2, 256], BF, tag="bcm")
        for t2 in range(2):
            accc = psm.tile([128, 512], F32, name=f"accc{t2}", tag=f"mmacc{t2 + 2}")
            for k in range(4):
                nc.tensor.matmul(accc[:, 0:256], dwk[:, 2 + t2, k, :],
                                 cvbc[:, t2, k + 1:257 + k], start=(k == 0),
                                 stop=(k == 3))
            nc.scalar.activation(bcm[:, t2, :], accc[:, 0:256], AF.Silu,
                                 bias=cbc[:, 2 + t2:3 + t2])
        agbc_in = dram.tile([256, 256], BF, tag="agbc_in")
        agbc_out = dram.tile([NCORES, 256, 256], BF, tag="agbc_out")
        for t2 in range(2):
            nc.sync.dma_start(agbc_in[128 * t2:128 * (t2 + 1), :], bcm[:, t2, :])
        nc.gpsimd.collective_compute(
            "AllGather", mybir.AluOpType.bypass,
            replica_groups=[list(range(NCORES))],
            ins=[agbc_in.opt()], outs=[agbc_out.opt()],
        )

        # rstd per token: sum of squares via ones-matmul, then Rsqrt
        s_bc = big.tile([128, TOK], F32, tag="s_bc")
        srow = big.tile([1, TOK], F32, tag="srow")
        for a in range(8):
            sq = t01.tile([128, TOK], F32R, tag="sq")
            nc.scalar.activation(sq[:], xTb[:, a, :], AF.Square)
            for n in range(4):
                nc.tensor.matmul(accs_q[n][:], onescol[:], sq[:, 512 * n:512 * (n + 1)],
                                 start=(a == 0), stop=(a == 7))
        srt = big.tile([1, TOK], F32, tag="srt")
        for n in range(4):
            nc.scalar.activation(srt[0:1, 512 * n:512 * (n + 1)], accs_q[n][:],
                                 AF.Sqrt, bias=epsc[0:1, 0:1], scale=1.0 / D)
        # broadcast sqrt first, then wide reciprocal (single-partition
        # reciprocal on [1,2048] costs ~13us; [128,2048] costs ~2us)
        nc.gpsimd.partition_broadcast(s_bc[:], srt[:], channels=128)
        nc.vector.reciprocal_approx_fast(out=s_bc[:], in_=s_bc[:])

        for mi, m in enumerate((6, 2, 3, 0, 1)):
            mm_m = 4 if m == 6 else 128
            accs = [psm.tile([128, 512], F32, name=f"mmacc{n}", tag=f"mmacc{n}")
                    for n in range(4)]
            for k in range(8):
                if m == 6:
                    lhs = wdt[:, k, :]
                else:
                    w_t = (wz, wx, wbc)[m // 2]
                    coff = (m % 2) * 128
                    lhs = w_t[:, k, coff:coff + 128]
                for n in range(4):
                    n0 = 512 * n
                    nc.tensor.matmul(accs[n][0:mm_m, :], lhs, xTb[:, k, n0:n0 + 512],
                                     start=(k == 0), stop=(k == 7))
            for n in range(4):
                n0 = 512 * n
                sb = s_bc[0:mm_m, n0:n0 + 512]
                if m < 2:
                    dst = zT[:, m, n0:n0 + 512]
                elif m < 6:
                    b = n0 // 1024
                    dst = cvin[:, m - 2, b, 4 + (n0 % 1024):4 + (n0 % 1024) + 512]
                else:
                    dst = dtraw[:, n0:n0 + 512]
                nc.vector.tensor_tensor(dst, accs[n][0:mm_m, :], sb, OP.mult)

    zx_dbg = dbg_out("zx_dbg", [768, TOK], BF)
    if zx_dbg is not None:
        nc.sync.dma_start(zx_dbg[0:256, :].rearrange("(a p) t -> p a t", p=128), zT[:])
        for mi in range(2):
            for b in range(2):
                nc.sync.dma_start(
                    zx_dbg[256 + 128 * mi:384 + 128 * mi, 1024 * b:1024 * (b + 1)],
                    cvin[:, mi, b, 4:1028])
    dtr_dbg = dbg_out("dtr_dbg", [HC, TOK], F32)
    if dtr_dbg is not None:
        nc.sync.dma_start(dtr_dbg[:], dtraw[:])
    if upto < 2:
        cv_stack.close(); mid_stack.close()
        return

    # ---------------- stage 2: conv+silu, dt/dA/cs ----------------
    ssd_stack = contextlib.ExitStack()
    ssd = ssd_stack.enter_context(tc.tile_pool(name="ssd", bufs=1))
    xsT = ssd.tile([128, 2, TOK], BF, tag="xsT")
    bT = ssd.tile([128, TOK], BF, tag="bT")
    cT = ssd.tile([128, TOK], BF, tag="cT")
    dtt = ssd.tile([HC, TOK], F32, tag="dtt")
    yT = ssd.tile([128, 2, TOK], BF, tag="yT")
    g = ssd.tile([128, 2, TOK], BF, tag="g")

    # x-part depthwise conv (B/C arrive via the early AllGather)
    nc.sync.dma_start(bT[:].rearrange("c (j t) -> c j t", j=8),
                      agbc_out[:, 0:128, :].rearrange("j c t -> c j t"))
    nc.sync.dma_start(cT[:].rearrange("c (j t) -> c j t", j=8),
                      agbc_out[:, 128:256, :].rearrange("j c t -> c j t"))
    with tc.tile_pool(name="cps", bufs=2, space="PSUM") as cps:
        for t in range(2):
            for b in range(2):
                for hv in range(2):
                    acc = cps.tile([128, 512], F32, tag="cacc")
                    o = 512 * hv
                    for k in range(4):
                        nc.tensor.matmul(acc[:], dwk[:, t, k, :],
                                         cvin[:, t, b, o + k + 1:o + 513 + k],
                                         start=(k == 0), stop=(k == 3))
                    bsl = slice(1024 * b + o, 1024 * b + o + 512)
                    nc.scalar.activation(xsT[:, t, bsl], acc[:], AF.Silu,
                                         bias=cbc[:, t:t + 1])

    dtb_c = pers.tile([HC, 1], F32, tag="dtb_c")
    nc.sync.dma_start(dtb_c[:], dt_bias[:])
    alog32_c = pers.tile([32, 1], F32, tag="alog32_c")
    nc.sync.dma_start(alog32_c[:], alog32[:])
    # softplus(x+b) = -ln(sigmoid(-(x+b)))
    dtbn = pers.tile([HC, 1], F32, tag="dtbn")
    nc.vector.tensor_scalar(out=dtbn[:], in0=dtb_c[:], scalar1=-1.0, scalar2=None,
                            op0=OP.mult)
    nc.scalar.activation(dtt[:], dtraw[:], AF.Sigmoid, scale=-1.0, bias=dtbn[:, 0:1])
    nc.scalar.activation(dtt[:], dtt[:], AF.Ln)
    nc.vector.tensor_scalar(out=dtt[:], in0=dtt[:], scalar1=-1.0, scalar2=None,
                            op0=OP.mult)

    dt_dbg = dbg_out("dt_dbg", [HC, TOK], F32)
    if dt_dbg is not None:
        nc.sync.dma_start(dt_dbg[:], dtt[:])
    xbc_dbg = dbg_out("xbc_dbg", [512, TOK], BF)
    if xbc_dbg is not None:
        nc.sync.dma_start(xbc_dbg[0:256, :].rearrange("(a p) t -> p a t", p=128), xsT[:])
        nc.sync.dma_start(xbc_dbg[256:384, :], bT[:])
        nc.sync.dma_start(xbc_dbg[384:512, :], cT[:])
    if upto < 3:
        ssd_stack.close(); cv_stack.close(); mid_stack.close()
        return

    # ---------------- stage 3: SSD ----------------
    # dAr (32 rows = h*8 + b*4 + c, 256): DMA from dtt then scale by -exp(A_log)
    dAr = ssd.tile([32, L], F32, tag="dAr")
    csr = ssd.tile([32, L], F32R, tag="csr")
    expcs = ssd.tile([32, L], F32R, tag="expcs")
    expa32 = ssd.tile([32, 1], F32, tag="expa32")
    nc.scalar.activation(expa32[:], alog32_c[:], AF.Exp)
    for bc in range(8):
        b, c = bc // 4, bc % 4
        nc.sync.dma_start(dAr[bc:bc + 25:8, :],
                          dtt[:, 1024 * b + 256 * c:1024 * b + 256 * (c + 1)])
    nc.vector.tensor_scalar(out=dAr[:], in0=dAr[:], scalar1=expa32[:, 0:1], scalar2=-1.0,
                            op0=OP.mult, op1=OP.mult)
    nc.vector.tensor_tensor_scan(csr[:], dAr[:], dAr[:], 0.0, OP.add, OP.bypass)
    nc.scalar.activation(expcs[:], csr[:], AF.Exp)
    decay_r = ssd.tile([32, L], F32, tag="decay_r")
    nc.scalar.activation(decay_r[:], csr[:], AF.Exp, scale=-1.0, bias=csr[:, L - 1:L])

    # prefetch tail weights (overlaps SSD compute; behind the staging DMAs)
    nc.sync.dma_start(wo_all[:], w_out.rearrange("(a p) m -> p a m", p=128))
    nc.sync.dma_start(wg_all[:], w_gate.rearrange("(a p) m -> p a m", p=128))
    nc.sync.dma_start(sc1_all[:], w_sc1.rearrange("(a p) m -> p a m", p=128))
    nc.sync.dma_start(wsm_all[:], w_summ.rearrange("(a p) m -> p a m", p=128))
    nc.sync.dma_start(wq_all[:], w_q.rearrange("(a p) m -> p a m", p=128))
    nc.sync.dma_start(wv_all[:], w_v.rearrange("s (a m) -> s a m", m=128))
    nc.sync.dma_start(wkt[:], w_k[:])
    nc.sync.dma_start(w2c[:], w_sc2.rearrange("(a p) o -> p (a o)", p=128))

    # s-major columns: dec as (128, 2st, 32r) with r = h*8+bc; dt per (b,c)
    dec_col = ssd.tile([128, 2, 32], F32, tag="dec_col")
    dt_col = ssd.tile([128, 2, 8, HC], F32, tag="dt_col")
    dtdec_col = ssd.tile([128, 2, 8, HC], F32, tag="dtdec_col")
    cs_colT = ssd.tile([128, 2, 32], F32, tag="cs_colT")
    with tc.tile_pool(name="psmt", bufs=2, space="PSUM") as psmt:
        for st in range(2):
            pt = psmt.tile([128, 32], F32, tag="mt32")
            nc.tensor.transpose(pt[:], decay_r[:, 128 * st:128 * (st + 1)],
                                ident[0:32, 0:32])
            nc.scalar.activation(dec_col[:, st, :], pt[:], AF.Copy)
            ptc = psmt.tile([128, 32], F32, tag="mt32c")
            nc.tensor.transpose(ptc[:].bitcast(F32R), csr[:, 128 * st:128 * (st + 1)],
                                identr[0:32, 0:32])
            nc.scalar.activation(cs_colT[:, st, :], ptc[:], AF.Copy)
            for bc in range(8):
                b, c = bc // 4, bc % 4
                pt2 = psmt.tile([128, HC], F32, tag="mt")
                t0 = 1024 * b + 256 * c + 128 * st
                nc.tensor.transpose(pt2[:], dtt[:, t0:t0 + 128], ident[0:HC, 0:HC])
                nc.scalar.activation(dt_col[:, st, bc, :], pt2[:], AF.Copy)
            for bc in range(8):
                nc.vector.tensor_tensor(dtdec_col[:, st, bc, :], dt_col[:, st, bc, :],
                                        dec_col[:, st, bc:bc + 25:8], OP.mult)

    # exp(cs_last) per row, broadcast to all partitions: etb [128, 32]
    etb = ssd.tile([128, 32], F32, tag="etb")
    with tc.tile_pool(name="pset", bufs=1, space="PSUM") as pset, \
         tc.tile_pool(name="tet", bufs=1) as tet:
        ptl = pset.tile([1, 32], F32, tag="ptl")
        nc.tensor.transpose(ptl[:].bitcast(F32R), csr[:, L - 1:L], identr[0:32, 0:32])
        etrow = tet.tile([1, 32], F32, tag="etrow")
        nc.scalar.activation(etrow[:], ptl[:], AF.Exp)
        nc.gpsimd.partition_broadcast(etb[:], etrow[:], channels=128)

    # mask bias: mb[p, st, l] = 0 if l >= 128*st+p else -1e30
    maskb = ssd.tile([128, 2, L], F32, tag="maskb")
    with tc.tile_pool(name="tio2", bufs=2) as tio2:
        for st in range(2):
            iol2 = tio2.tile([128, L], F32, tag="iol2")
            nc.gpsimd.iota(iol2[:], pattern=[[1, L]], base=-128 * st,
                           channel_multiplier=-1, allow_small_or_imprecise_dtypes=True)
            nc.vector.tensor_scalar(out=maskb[:, st, :], in0=iol2[:], scalar1=0.0,
                                    scalar2=-1e30, op0=OP.is_lt, op1=OP.mult)

    # D per pair-of-heads column (pre-broadcast on host)
    dDc = pers.tile([128, 2], F32, tag="dDc")
    nc.sync.dma_start(dDc[:], d_in[:])


    a2a_in = dram.tile([NCORES, CH, MYTOK], BF, tag="a2a_in")
    a2a_out = dram.tile([NCORES, CH, MYTOK], BF, tag="a2a_out")

    Rr = ssd.tile([128, 2, CH], BF, tag="Rr")
    with tc.tile_pool(name="psT", bufs=1, space="PSUM") as psT, \
         tc.tile_pool(name="psS2", bufs=1, space="PSUM") as psS2, \
         tc.tile_pool(name="psG", bufs=1, space="PSUM") as psG, \
         tc.tile_pool(name="psY", bufs=2, space="PSUM") as psY, \
         tc.tile_pool(name="psB", bufs=1, space="PSUM") as psB, \
         tc.tile_pool(name="tT", bufs=2) as tT, \
         tc.tile_pool(name="tG", bufs=3) as tG, \
         tc.tile_pool(name="tM", bufs=3) as tM:
        for b in range(2):
            for c in range(4):
                bc = b * 4 + c
                t0 = 1024 * b + 256 * c
                # csr/expcs rows for this chunk staged at partitions {0,64}
                # (PE base-partition alignment); scalar-queue DMAs so they
                # don't queue behind the big weight prefetch
                cs4a = tT.tile([128, L], F32R, tag="cs4a")
                cs4b = tT.tile([128, L], F32R, tag="cs4b")
                nc.scalar.dma_start(cs4a[0:65:64, :], csr[bc:bc + 9:8, :])
                nc.scalar.dma_start(cs4b[0:65:64, :], csr[bc + 16:bc + 25:8, :])
                if c > 0:
                    ec4a = tT.tile([128, L], F32R, tag="ec4a")
                    ec4b = tT.tile([128, L], F32R, tag="ec4b")
                    nc.scalar.dma_start(ec4a[0:65:64, :], expcs[bc:bc + 9:8, :])
                    nc.scalar.dma_start(ec4b[0:65:64, :], expcs[bc + 16:bc + 25:8, :])
                # per-chunk transposes: xd/xdd (s-major), bS
                xd = tT.tile([128, 2, CH], BF, tag="xd")
                xdd = tT.tile([128, 2, CH], BF, tag="xdd")
                bS = tT.tile([128, 2, NSTATE], BF, tag="bS")
                for st in range(2):
                    ts0 = t0 + 128 * st
                    for cb in range(2):
                        pt = psT.tile([128, 128], BF, tag="xdt")
                        nc.tensor.transpose(pt[:], xsT[:, cb, ts0:ts0 + 128], identb[:])
                        for hh in range(2):
                            h = 2 * cb + hh
                            nc.vector.tensor_scalar(
                                out=xd[:, st, 64 * h:64 * (h + 1)],
                                in0=pt[:, 64 * hh:64 * (hh + 1)],
                                scalar1=dt_col[:, st, bc, h:h + 1], scalar2=None,
                                op0=OP.mult)
                            nc.vector.tensor_scalar(
                                out=xdd[:, st, 64 * h:64 * (h + 1)],
                                in0=pt[:, 64 * hh:64 * (hh + 1)],
                                scalar1=dtdec_col[:, st, bc, h:h + 1], scalar2=None,
                                op0=OP.mult)
                    ptb = psT.tile([128, 128], BF, tag="bdt")
                    nc.tensor.transpose(ptb[:], bT[:, ts0:ts0 + 128], identb[:])
                    nc.scalar.activation(bS[:, st, :], ptb[:], AF.Copy)
                # states S (n, (h,p))
                sps = psS2.tile([128, CH], F32, tag="sps")
                for st in range(2):
                    nc.tensor.matmul(sps[:], bS[:, st, :], xdd[:, st, :],
                                     start=(st == 0), stop=(st == 1))
                S = tT.tile([128, CH], F32, tag="S")
                nc.scalar.activation(S[:], sps[:], AF.Copy)
                # Gt (s,l) shared across heads
                gts = []
                for st in range(2):
                    pg = psG.tile([128, L], F32, tag="pg")
                    nc.tensor.matmul(pg[:], bT[:, t0 + 128 * st:t0 + 128 * (st + 1)],
                                     cT[:, t0:t0 + 256], start=True, stop=True)
                    gt = tG.tile([128, L], BF, tag="gt")
                    nc.scalar.activation(gt[:], pg[:], AF.Copy)
                    gts.append(gt)
                for h in range(HC):
                    cb, hh = h // 2, h % 2
                    r = h * 8 + bc
                    hp = 64 * (h % 2)
                    cs4 = cs4a if h < 2 else cs4b
                    # broadcast csr row r to all partitions via rank-1 matmul
                    csp = psB.tile([128, L], F32, tag="csp")
                    nc.tensor.matmul(csp[:], onesb[hp:hp + 1, :], cs4[hp:hp + 1, :],
                                     start=True, stop=True)
                    ce = None
                    if c > 0:
                        ec4 = ec4a if h < 2 else ec4b
                        ecp = psB.tile([128, L], F32, tag="ecp")
                        nc.tensor.matmul(ecp[:], onesb[hp:hp + 1, :], ec4[hp:hp + 1, :],
                                         start=True, stop=True)
                        ce = tM.tile([128, L], BF, tag="ce")
                        nc.vector.tensor_tensor(ce[:], cT[:, t0:t0 + 256], ecp[:],
                                                OP.mult)
                    psl = slice(64 * hh, 64 * (hh + 1))
                    ypb = psY.tile([64, L], F32, tag="ypb")
                    ltp2 = tM.tile([128, 2, L], F32, tag="ltp2")
                    for st in range(2):
                        nc.vector.scalar_tensor_tensor(
                            out=ltp2[:, st, :], in0=csp[:],
                            scalar=cs_colT[:, st, r:r + 1],
                            in1=maskb[:, st, :], op0=OP.subtract, op1=OP.add)
                    lt2 = tM.tile([128, 2, L], BF, tag="lt2")
                    nc.scalar.activation(lt2[:], ltp2[:], AF.Exp)
                    for st in range(2):
                        ms = tM.tile([128, L], BF, tag="ms")
                        nc.gpsimd.tensor_tensor(ms[:], lt2[:, st, :], gts[st][:],
                                                OP.mult)
                        nc.tensor.matmul(ypb[:], xd[:, st, 64 * h:64 * (h + 1)],
                                         ms[:], start=(st == 0),
                                         stop=(st == 1 and c == 0))
                    if c > 0:
                        nc.tensor.matmul(ypb[:], Rr[:, b, 64 * h:64 * (h + 1)],
                                         ce[:], start=False, stop=True)
                    nc.vector.scalar_tensor_tensor(
                        out=yT[psl, cb, t0:t0 + 256], in0=xsT[psl, cb, t0:t0 + 256],
                        scalar=dDc[psl, cb:cb + 1], in1=ypb[0:64, :],
                        op0=OP.mult, op1=OP.add)
                # R update for next chunk
                if c == 0:
                    nc.scalar.activation(Rr[:, b, :], S[:], AF.Copy)
                elif c < 3:
                    for h in range(HC):
                        r = h * 8 + bc
                        nc.vector.scalar_tensor_tensor(
                            out=Rr[:, b, 64 * h:64 * (h + 1)],
                            in0=Rr[:, b, 64 * h:64 * (h + 1)],
                            scalar=etb[:, r:r + 1],
                            in1=S[:, 64 * h:64 * (h + 1)], op0=OP.mult, op1=OP.add)
                # gated z*silu for this chunk + A2A staging (dest core j == bc)
                for cb in range(2):
                    sz = tM.tile([128, 256], BF, tag="sz")
                    nc.scalar.activation(sz[:], zT[:, cb, t0:t0 + 256], AF.Silu)
                    nc.gpsimd.tensor_tensor(g[:, cb, t0:t0 + 256],
                                            yT[:, cb, t0:t0 + 256], sz[:], OP.mult)
                    nc.sync.dma_start(a2a_in[bc, 128 * cb:128 * (cb + 1), :],
                                      g[:, cb, t0:t0 + 256])

    y_dbg = dbg_out("y_dbg", [CH, TOK], BF)
    if y_dbg is not None:
        nc.sync.dma_start(y_dbg.rearrange("(a p) t -> p a t", p=128), yT[:])
    g_dbg = dbg_out("g_dbg", [CH, TOK], BF)
    if g_dbg is not None:
        nc.sync.dma_start(g_dbg.rearrange("(a p) t -> p a t", p=128), g[:])
    if upto < 5:
        ssd_stack.close(); cv_stack.close(); mid_stack.close()
        return

    # ---------------- stage 5: AllToAll ----------------
    ssd_stack.close()
    cv_stack.close()
    mid_stack.close()
    nc.gpsimd.collective_compute(
        "AllToAll", mybir.AluOpType.bypass,
        replica_groups=[list(range(NCORES))],
        ins=[a2a_in.opt()], outs=[a2a_out.opt()],
    )
    st6 = ES.enter_context(tc.tile_pool(name="st6", bufs=1))
    g2 = st6.tile([128, 16, MYTOK], BF, tag="g2")
    nc.sync.dma_start(g2[:], a2a_out.rearrange("j (cb p) t -> p (j cb) t", p=128))

    # ---------------- stage 5b: gated RMSNorm (token-local) ----------------
    # ssm_norm_w is folded into w_out on the host, so yn = g2 * rstd only.
    yn = st6.tile([128, 16, MYTOK], BF, tag="yn")
    rstd_bc = st6.tile([128, MYTOK], F32, tag="rstd_bc")
    with tc.tile_pool(name="tn", bufs=3) as tn, \
         tc.tile_pool(name="psn", bufs=1, space="PSUM") as psn:
        ssps = psn.tile([1, MYTOK], F32, tag="ssps")
        for i in range(16):
            gsq = tn.tile([128, MYTOK], BF, tag="gsq")
            nc.scalar.activation(gsq[:], g2[:, i, :], AF.Square)
            nc.tensor.matmul(ssps[:], onescolb[:], gsq[:], start=(i == 0),
                             stop=(i == 15))
        rstd_s = tn.tile([1, MYTOK], F32, tag="rstd_s")
        nc.scalar.activation(rstd_s[:], ssps[:], AF.Sqrt, bias=epsc[0:1, 0:1],
                             scale=1.0 / DI)
        nc.gpsimd.partition_broadcast(rstd_bc[:], rstd_s[:], channels=128)
        nc.vector.reciprocal_approx_fast(out=rstd_bc[:], in_=rstd_bc[:])
    for i in range(16):
        nc.vector.tensor_tensor(yn[:, i, :], g2[:, i, :], rstd_bc[:], OP.mult)
    yn_dbg = dbg_out("yn_dbg", [DI, MYTOK], BF)
    if yn_dbg is not None:
        nc.sync.dma_start(yn_dbg.rearrange("(a p) t -> p a t", p=128), yn[:])
    if upto < 6:
        return

    # ---------------- stage 6a: out_proj, scorer, summaries, q ----------------
    y2 = st6.tile([128, 8, MYTOK], BF, tag="y2")
    with tc.tile_pool(name="ps6", bufs=2, space="PSUM") as ps6:
        for m in range(8):
            acc = ps6.tile([128, MYTOK], F32, tag="oacc")
            for k in range(16):
                nc.tensor.matmul(acc[:], wo_all[:, k, 128 * m:128 * (m + 1)], yn[:, k, :],
                                 start=(k == 0), stop=(k == 15))
            if m % 2 == 0:
                nc.vector.tensor_copy(y2[:, m, :], acc[:])
            else:
                nc.scalar.activation(y2[:, m, :], acc[:], AF.Copy)

        # scorer
        rl1 = st6.tile([128, 2, MYTOK], BF, tag="rl1")
        for m in range(2):
            acc = ps6.tile([128, MYTOK], F32, tag="oacc")
            for k in range(8):
                nc.tensor.matmul(acc[:], sc1_all[:, k, 128 * m:128 * (m + 1)], y2[:, k, :],
                                 start=(k == 0), stop=(k == 7))
            nc.scalar.activation(rl1[:, m, :], acc[:], AF.Relu)
        u_row = st6.tile([1, MYTOK], F32, tag="u_row")
        ups = ps6.tile([1, MYTOK], F32, tag="ups")
        for m in range(2):
            nc.tensor.matmul(ups[:], w2c[:, m:m + 1], rl1[:, m, :], start=(m == 0),
                             stop=(m == 1))
        nc.vector.tensor_copy(u_row[:], ups[:])

        # summaries + q (64-col projections of y2)
        summT = st6.tile([64, MYTOK], BF, tag="summT")
        qT = st6.tile([64, MYTOK], BF, tag="qT")
        for (wv_t, dst) in ((wsm_all, summT), (wq_all, qT)):
            acc = ps6.tile([64, MYTOK], F32, tag="sacc6")
            for k in range(8):
                nc.tensor.matmul(acc[:], wv_t[:, k, :], y2[:, k, :], start=(k == 0),
                                 stop=(k == 7))
            nc.scalar.activation(dst[:], acc[:], AF.Copy)

        # summaries token-major (for allgather), as f32
        stm = st6.tile([128, 2, 64], F32, tag="stm")
        for st in range(2):
            pt = ps6.tile([128, 64], BF, tag="stp")
            nc.tensor.transpose(pt[:], summT[:, 128 * st:128 * (st + 1)],
                                identb[0:64, 0:64])
            nc.scalar.activation(stm[:, st, :], pt[:], AF.Copy)

    # gate phase 1: y2 half (overlaps the pool-selection collectives)
    gy2 = st6.tile([128, 8, MYTOK], F32, tag="gy2")
    with tc.tile_pool(name="psg1", bufs=2, space="PSUM") as psg1:
        for m in range(8):
            acc = psg1.tile([128, MYTOK], F32, tag="g1acc")
            for k in range(8):
                nc.tensor.matmul(acc[:], wg_all[:, k, 128 * m:128 * (m + 1)], y2[:, k, :],
                                 start=(k == 0), stop=(k == 7))
            nc.scalar.activation(gy2[:, m, :], acc[:], AF.Copy)

    y2_dbg = dbg_out("y2_dbg", [D, MYTOK], BF)
    if y2_dbg is not None:
        nc.sync.dma_start(y2_dbg.rearrange("(a p) t -> p a t", p=128), y2[:])
    u_dbg = dbg_out("u_dbg", [1, MYTOK], F32)
    if u_dbg is not None:
        nc.sync.dma_start(u_dbg[:], u_row[:])
    if upto < 7:
        return

    # ---------------- stage 6b: allgather u+summaries, ranks, members ----------------
    ag1_in = dram.tile([MYTOK, 65], F32, tag="ag1_in")
    ag1_out = dram.tile([NCORES, MYTOK, 65], F32, tag="ag1_out")
    nc.sync.dma_start(ag1_in[:, 0:1].rearrange("t o -> o t"), u_row[:])
    for st in range(2):
        nc.sync.dma_start(ag1_in[128 * st:128 * (st + 1), 1:65], stm[:, st, :])
    nc.gpsimd.collective_compute(
        "AllGather", mybir.AluOpType.bypass,
        replica_groups=[list(range(NCORES))],
        ins=[ag1_in.opt()], outs=[ag1_out.opt()],
    )
    u_all = st6.tile([1, TOK], F32, tag="u_all")
    nc.sync.dma_start(u_all[:], ag1_out[:, :, 0:1].rearrange("j t o -> o (j t)"))
    summ_all = st6.tile([128, 16, 64], F32, tag="summ_all")
    summ_allr = st6.tile([128, 16, 64], BF, tag="summ_allr")
    nc.sync.dma_start(summ_all[:],
                      ag1_out[:, :, 1:65].rearrange("j (st p) c -> p (j st) c", p=128))

    # ranks for my 256 tokens vs all 2048
    u_bc = st6.tile([128, TOK], F32, tag="u_bc")
    nc.gpsimd.partition_broadcast(u_bc[:], u_all[:], channels=128)
    iota_all = st6.tile([128, TOK], F32, tag="iota_all")
    nc.gpsimd.iota(iota_all[:], pattern=[[1, TOK]], base=0, channel_multiplier=0,
                   allow_small_or_imprecise_dtypes=True)
    u_col = st6.tile([128, 2], F32, tag="u_col")
    with tc.tile_pool(name="psu", bufs=2, space="PSUM") as psu:
        for st in range(2):
            pu = psu.tile([128, 1], F32, tag="pu")
            nc.tensor.transpose(pu[:], u_row[0:1, 128 * st:128 * (st + 1)],
                                ident[0:1, 0:1])
            nc.vector.tensor_copy(u_col[:, st:st + 1], pu[:])
    tid_col = st6.tile([128, 2], F32, tag="tid_col")
    nc.sync.dma_start(tid_col[:], tok_ids.rearrange("(a p) o -> p (a o)", p=128))
    member = st6.tile([128, 2], F32, tag="member")
    with tc.tile_pool(name="trk", bufs=1) as trk:
        for st in range(2):
            junk = trk.tile([128, TOK], F32, tag="junk")
            rgt = trk.tile([128, 1], F32, tag="rgt")
            nc.vector.tensor_scalar(out=junk[:], in0=u_bc[:], scalar1=u_col[:, st:st + 1],
                                    scalar2=0.0, op0=OP.is_gt, op1=OP.add,
                                    accum_out=rgt[:])
            eqm = trk.tile([128, TOK], F32, tag="eqm")
            nc.vector.tensor_scalar(out=eqm[:], in0=u_bc[:], scalar1=u_col[:, st:st + 1],
                                    scalar2=None, op0=OP.is_equal)
            junk2 = trk.tile([128, TOK], F32, tag="junk2")
            req = trk.tile([128, 1], F32, tag="req")
            nc.vector.scalar_tensor_tensor(out=junk2[:], in0=iota_all[:],
                                           scalar=tid_col[:, st:st + 1], in1=eqm[:],
                                           op0=OP.is_lt, op1=OP.mult, accum_out=req[:])
            rank = trk.tile([128, 1], F32, tag="rank")
            nc.vector.tensor_tensor(rank[:], rgt[:], req[:], OP.add)
            rlt = trk.tile([128, 1], F32, tag="rlt")
            nc.vector.tensor_scalar(out=rlt[:], in0=rank[:], scalar1=float(KRANK),
                                    scalar2=None, op0=OP.is_lt)
            vld = trk.tile([128, 1], F32, tag="vld")
            nc.vector.tensor_scalar(out=vld[:], in0=u_col[:, st:st + 1], scalar1=0.0,
                                    scalar2=None, op0=OP.is_gt)
            nc.vector.tensor_tensor(member[:, st:st + 1], rlt[:], vld[:], OP.mult)

    ag2_in = dram.tile([MYTOK, 1], F32, tag="ag2_in")
    ag2_out = dram.tile([NCORES, MYTOK, 1], F32, tag="ag2_out")
    for st in range(2):
        nc.sync.dma_start(ag2_in[128 * st:128 * (st + 1), :], member[:, st:st + 1])
    nc.gpsimd.collective_compute(
        "AllGather", mybir.AluOpType.bypass,
        replica_groups=[list(range(NCORES))],
        ins=[ag2_in.opt()], outs=[ag2_out.opt()],
    )

    # ---------------- stage 6c: retrieval (logits/exp overlap AG2) ----------------
    summT_all = st6.tile([64, TOK], BF, tag="summT_all")
    esm = st6.tile([128, 16, MYTOK], BF, tag="esm")
    retrT = st6.tile([128, 8, MYTOK], BF, tag="retrT")
    # cond from u_all (overlaps AG2 as well)
    cond_col = st6.tile([128, 1], F32, tag="cond_col")
    with tc.tile_pool(name="tcd", bufs=1) as tcd:
        sgj = tcd.tile([1, TOK], F32, tag="sgj")
        sgs = tcd.tile([1, 1], F32, tag="sgs")
        nc.scalar.activation(sgj[:], u_all[:], AF.Sigmoid, accum_out=sgs[:])
        vj = tcd.tile([1, TOK], F32, tag="vj")
        vs = tcd.tile([1, 1], F32, tag="vs")
        nc.vector.tensor_scalar(out=vj[:], in0=u_all[:], scalar1=0.0, scalar2=0.0,
                                op0=OP.is_gt, op1=OP.add, accum_out=vs[:])
        c1 = tcd.tile([1, 1], F32, tag="c1")
        nc.vector.tensor_scalar(out=c1[:], in0=sgs[:], scalar1=float(TAU2 * TOK),
                                scalar2=None, op0=OP.is_gt)
        c2 = tcd.tile([1, 1], F32, tag="c2")
        nc.vector.tensor_scalar(out=c2[:], in0=vs[:], scalar1=0.0, scalar2=None,
                                op0=OP.is_gt)
        cnd = tcd.tile([1, 1], F32, tag="cnd")
        nc.vector.tensor_tensor(cnd[:], c1[:], c2[:], OP.mult)
        nc.gpsimd.partition_broadcast(cond_col[:], cnd[:], channels=128)

    xres = st6.tile([128, 8, MYTOK], F32, tag="xres")
    nc.sync.dma_start(xres[:], x_myT.rearrange("(a p) t -> p a t", p=128))

    with tc.tile_pool(name="tr6", bufs=4) as tr6, \
         tc.tile_pool(name="psr6", bufs=1, space="PSUM") as psr6, \
         tc.tile_pool(name="psl6", bufs=2, space="PSUM") as psl6, \
         tc.tile_pool(name="psrp", bufs=1, space="PSUM") as psrp, \
         tc.tile_pool(name="psq6", bufs=1, space="PSUM") as psq6:
        for i in range(16):
            pt = psq6.tile([64, 128], F32, tag="satp")
            nc.tensor.transpose(pt[:], summ_all[:, i, :], ident[:])
            nc.scalar.activation(summT_all[:, 128 * i:128 * (i + 1)], pt[:], AF.Copy)
        wktp = psr6.tile([64, 64], BF, tag="wktp")
        nc.tensor.transpose(wktp[:], wkt[:], identb[0:64, 0:64])
        kwT = tr6.tile([64, 64], BF, tag="kwT")
        nc.scalar.activation(kwT[:], wktp[:], AF.Copy)
        kqp = psr6.tile([64, MYTOK], F32, tag="kqp")
        nc.tensor.matmul(kqp[:], kwT[:], qT[:], start=True, stop=True)
        kq = tr6.tile([64, MYTOK], BF, tag="kq")
        nc.vector.tensor_scalar(out=kq[:], in0=kqp[:], scalar1=0.25, scalar2=None,
                                op0=OP.mult)
        # unmasked exp(logits) — mask applied multiplicatively after AG2
        for i in range(16):
            lp = psl6.tile([128, MYTOK], F32, tag="lp")
            nc.tensor.matmul(lp[:], summT_all[:, 128 * i:128 * (i + 1)], kq[:],
                             start=True, stop=True)
            nc.scalar.activation(esm[:, i, :], lp[:], AF.Exp)
        mask_col = st6.tile([128, 16], F32, tag="mask_col")
        nc.sync.dma_start(mask_col[:], ag2_out.rearrange("j (a p) o -> p (j a o)", p=128))
        mem_dbg = dbg_out("mem_dbg", [1, TOK], F32)
        if mem_dbg is not None:
            nc.sync.dma_start(mem_dbg[:], ag2_out.rearrange("j t o -> o (j t)"))
        # masked bf16 summaries (mask folded into the f32->bf16 cast) and
        # softmax denominator as a mask-vector matmul on the tensor engine
        maskb16 = tr6.tile([128, 16], BF, tag="maskb16")
        nc.vector.tensor_scalar(out=maskb16[:], in0=mask_col[:], scalar1=1.0,
                                scalar2=None, op0=OP.mult)
        for i in range(16):
            nc.scalar.activation(summ_allr[:, i, :], summ_all[:, i, :], AF.Copy,
                                 scale=mask_col[:, i:i + 1])
        denps = psr6.tile([1, MYTOK], F32, tag="denps")
        for i in range(16):
            nc.tensor.matmul(denps[:], maskb16[:, i:i + 1], esm[:, i, :],
                             start=(i == 0), stop=(i == 15))
        den_row = tr6.tile([1, MYTOK], F32, tag="den_row")
        nc.scalar.activation(den_row[:], denps[:], AF.Copy)
        rden_bc = tr6.tile([64, MYTOK], F32, tag="rden_bc")
        nc.gpsimd.partition_broadcast(rden_bc[:], den_row[:], channels=64)
        nc.vector.reciprocal_approx_fast(out=rden_bc[:], in_=rden_bc[:])
        tmpp = psr6.tile([64, MYTOK], F32, tag="tmpp")
        for i in range(16):
            nc.tensor.matmul(tmpp[:], summ_allr[:, i, :], esm[:, i, :], start=(i == 0),
                             stop=(i == 15))
        tmps = tr6.tile([64, MYTOK], BF, tag="tmps")
        nc.vector.tensor_tensor(tmps[:], tmpp[:], rden_bc[:], OP.mult)
        for m in range(8):
            rp = psrp.tile([128, MYTOK], F32, tag="rp")
            nc.tensor.matmul(rp[:], wv_all[:, m, :], tmps[:], start=True, stop=True)
            if m % 2 == 0:
                nc.vector.tensor_copy(retrT[:, m, :], rp[:])
            else:
                nc.scalar.activation(retrT[:, m, :], rp[:], AF.Copy)

    retr_dbg = dbg_out("retr_dbg", [D, MYTOK], BF)
    if retr_dbg is not None:
        nc.sync.dma_start(retr_dbg.rearrange("(a p) t -> p a t", p=128), retrT[:])
    if upto < 9:
        return

    # ---------------- stage 6d: gate, final ----------------
    with tc.tile_pool(name="psg6", bufs=3, space="PSUM") as psg6, \
         tc.tile_pool(name="tf6", bufs=3) as tf6:
        for m in range(8):
            acc = psg6.tile([128, MYTOK], F32, tag="gacc")
            for k in range(8, 16):
                nc.tensor.matmul(acc[:], wg_all[:, k, 128 * m:128 * (m + 1)],
                                 retrT[:, k - 8, :], start=(k == 8), stop=(k == 15))
            gl = tf6.tile([128, MYTOK], F32, tag="gl")
            nc.vector.tensor_tensor(gl[:], acc[:], gy2[:, m, :], OP.add)
            gsb = tf6.tile([128, MYTOK], F32, tag="gsb")
            nc.scalar.activation(gsb[:], gl[:], AF.Sigmoid)
            t1 = tf6.tile([128, MYTOK], F32, tag="t1")
            nc.vector.tensor_tensor(t1[:], gsb[:], retrT[:, m, :], OP.mult)
            t2 = tf6.tile([128, MYTOK], F32, tag="t2")
            nc.vector.scalar_tensor_tensor(out=t2[:], in0=t1[:], scalar=cond_col[:, 0:1],
                                           in1=y2[:, m, :], op0=OP.mult, op1=OP.add)
            fin = tf6.tile([128, MYTOK], F32, tag="fin")
            nc.vector.tensor_tensor(fin[:], t2[:], xres[:, m, :], OP.add)
            nc.sync.dma_start(out_my[128 * m:128 * (m + 1), :], fin[:])


# ---- host-side sharding ----


def _bc_window(x_padT, k):
    # conv halo for this core's 256 tokens; zeroed at batch boundaries
    # (reference pads the causal conv per batch: batch 1 starts at token 1024)
    w = x_padT[:, 256 * k:256 * k + 260].copy()
    if k % 4 == 0:
        w[:, 0:4] = 0
    return np.ascontiguousarray(w)


def make_in_maps(inputs):
    import ml_dtypes
    BF_NP = ml_dtypes.bfloat16
    x = np.asarray(inputs['x'], np.float32)
    x_tok = np.ascontiguousarray(x.reshape(2048, 1024))
    x_bf = np.ascontiguousarray(x_tok.T.astype(BF_NP))
    x_padT = np.pad(x_tok.T.astype(np.float32), ((0, 0), (4, 0))).astype(BF_NP)
    ipw = (np.asarray(inputs['in_proj_w'], np.float32)
           * np.asarray(inputs['norm_w'], np.float32)[:, None])
    cw = np.asarray(inputs['conv_w'], np.float32)
    cb = np.asarray(inputs['conv_b'], np.float32)
    w_out = np.ascontiguousarray(
        (np.asarray(inputs['ssm_norm_w'], np.float32)[:, None]
         * np.asarray(inputs['out_proj_w'], np.float32)).astype(BF_NP))
    w_sc1 = np.ascontiguousarray(np.asarray(inputs['scorer_w1'], np.float32).astype(BF_NP))
    w_sc2 = np.ascontiguousarray(np.asarray(inputs['scorer_w2'], np.float32).astype(BF_NP))
    w_summ = np.ascontiguousarray(np.asarray(inputs['summ_w'], np.float32).astype(BF_NP))
    w_q = np.ascontiguousarray(np.asarray(inputs['q_w'], np.float32).astype(BF_NP))
    w_k = np.ascontiguousarray(np.asarray(inputs['k_w'], np.float32).astype(BF_NP))
    w_v = np.ascontiguousarray(np.asarray(inputs['v_w'], np.float32).astype(BF_NP))
    w_gate = np.ascontiguousarray(np.asarray(inputs['gate_w'], np.float32).astype(BF_NP))
    in_maps = []
    for k in range(8):
        zc = ipw[:, 256 * k:256 * (k + 1)]
        xc = ipw[:, 2048 + 256 * k:2048 + 256 * (k + 1)]
        bcc = ipw[:, 4096:4352]
        dtc = ipw[:, 4352 + 4 * k:4352 + 4 * (k + 1)]
        w_my = np.ascontiguousarray(
            np.concatenate([zc, xc, bcc, dtc], axis=1).astype(BF_NP))
        conv_rows = np.concatenate([cw[256 * k:256 * (k + 1)], cw[2048:2304]], axis=0)
        convb_rows = np.concatenate([cb[256 * k:256 * (k + 1)], cb[2048:2304]], axis=0)
        m = {
            'x_bf': x_bf,
            'x_bc': _bc_window(x_padT, k),
            'w_in': w_my,
            'conv_w': np.ascontiguousarray(conv_rows),
            'conv_b': np.ascontiguousarray(convb_rows),
            'dt_bias': np.ascontiguousarray(inputs['dt_bias'][4 * k:4 * (k + 1), None]).astype(np.float32),
            'alog32': np.ascontiguousarray(np.repeat(inputs['A_log'][4 * k:4 * (k + 1)], 8)[:, None]).astype(np.float32),
            'd_in': np.ascontiguousarray(
                np.stack([np.repeat(inputs['D'][4 * k:4 * k + 2], 64),
                          np.repeat(inputs['D'][4 * k + 2:4 * k + 4], 64)], axis=1)).astype(np.float32),
            'w_out': w_out,
            'w_sc1': w_sc1,
            'w_sc2': w_sc2,
            'w_summ': w_summ,
            'w_q': w_q,
            'w_k': w_k,
            'w_v': w_v,
            'w_gate': w_gate,
            'x_myT': np.ascontiguousarray(x_tok[256 * k:256 * (k + 1), :].T),
            'tok_ids': np.arange(256 * k, 256 * (k + 1), dtype=np.float32)[:, None],
        }
        in_maps.append(m)
    return in_maps


def gather_out(results):
    out = np.empty((2048, 1024), np.float32)
    for k in range(8):
        out[256 * k:256 * (k + 1), :] = results[k]['out_my'].T
    return out.reshape(2, 1024, 1024)


_CACHED = {}


def _get_nc():
    if "nc" not in _CACHED:
        _CACHED["nc"] = build(upto=9, debug=False)[0]
    return _CACHED["nc"]


def kernel(**inputs):
    from concourse import bass_utils
    nc = _get_nc()
    in_maps = make_in_maps(inputs)
    res = bass_utils.run_bass_kernel_spmd(nc, in_maps, core_ids=list(range(NCORES)))
    return gather_out(res.results)


# revision 70
# speedup vs baseline: 1.0675x; 1.0385x over previous
"""MemMambaBlock Trainium2 kernel (self-contained).

8-core SPMD: head-sharded in_proj/conv/SSD -> AllToAll -> token-sharded
norm/out_proj/scorer/top-50-pool/retrieval/gate. The sequential memory-pool
scan is replaced by an exact parallel top-50 selection (streaming top-k
equivalence; slot order is irrelevant because the masked softmax retrieval
is permutation-invariant over pool slots).

v1: bf16 weights/activations, prefetched tail weights, PE-broadcast decay
matrices (no vector scans), vector/gpsimd split, per-chunk A2A staging.
"""
import contextlib
import numpy as np
import concourse.bass as bass
import concourse.bacc as bacc
import concourse.mybir as mybir
import concourse.tile as tile
from concourse.alu_op_type import AluOpType as OP

AF = mybir.ActivationFunctionType
F32 = mybir.dt.float32
F32R = mybir.dt.float32r
BF = mybir.dt.bfloat16
ROP = bass.bass_isa.ReduceOp

NCORES = 8
TOK = 2048
D = 1024
DI = 2048
HC = 4
CH = HC * 64          # 256
L = 256
NSTATE = 128
MYTOK = TOK // NCORES  # 256
WCOLS = 2 * CH + 2 * NSTATE + HC  # 772
EPS = 1e-5
TAU2 = 0.3
KRANK = 50


def build(upto=9, debug=False):
    nc = bacc.Bacc("TRN2", target_bir_lowering=False, debug=False, num_devices=NCORES)

    # ---------------- DRAM I/O ----------------
    x_bf = nc.dram_tensor("x_bf", [D, TOK], BF, kind="ExternalInput").ap()
    w_in = nc.dram_tensor("w_in", [D, WCOLS], BF, kind="ExternalInput").ap()
    conv_w = nc.dram_tensor("conv_w", [512, 4], F32, kind="ExternalInput").ap()
    conv_b = nc.dram_tensor("conv_b", [512], F32, kind="ExternalInput").ap()
    dt_bias = nc.dram_tensor("dt_bias", [HC, 1], F32, kind="ExternalInput").ap()
    alog32 = nc.dram_tensor("alog32", [32, 1], F32, kind="ExternalInput").ap()
    d_in = nc.dram_tensor("d_in", [128, 2], F32, kind="ExternalInput").ap()
    w_out = nc.dram_tensor("w_out", [DI, D], BF, kind="ExternalInput").ap()
    w_sc1 = nc.dram_tensor("w_sc1", [D, 256], BF, kind="ExternalInput").ap()
    w_sc2 = nc.dram_tensor("w_sc2", [256, 1], BF, kind="ExternalInput").ap()
    w_summ = nc.dram_tensor("w_summ", [D, 64], BF, kind="ExternalInput").ap()
    w_q = nc.dram_tensor("w_q", [D, 64], BF, kind="ExternalInput").ap()
    w_k = nc.dram_tensor("w_k", [64, 64], BF, kind="ExternalInput").ap()
    w_v = nc.dram_tensor("w_v", [64, 1024], BF, kind="ExternalInput").ap()
    w_gate = nc.dram_tensor("w_gate", [DI, D], BF, kind="ExternalInput").ap()
    x_bc = nc.dram_tensor("x_bc", [D, 260], BF, kind="ExternalInput").ap()
    x_myT = nc.dram_tensor("x_myT", [D, MYTOK], F32, kind="ExternalInput").ap()
    tok_ids = nc.dram_tensor("tok_ids", [MYTOK, 1], F32, kind="ExternalInput").ap()
    out_my = nc.dram_tensor("out_my", [D, MYTOK], F32, kind="ExternalOutput").ap()

    dbg = {}

    def dbg_out(name, shape, dt=F32, cond=True):
        if debug and cond:
            dbg[name] = nc.dram_tensor(name, shape, dt, kind="ExternalOutput").ap()
            return dbg[name]
        return None

    with tile.TileContext(nc) as tc, contextlib.ExitStack() as ES, \
            nc.allow_low_precision(reason="bf16 kernel validated vs fp32 ref"):
        _body(nc, tc, ES, upto, dbg_out, dict(
            x_bf=x_bf, w_in=w_in, conv_w=conv_w, conv_b=conv_b,
            dt_bias=dt_bias, alog32=alog32, d_in=d_in,
            w_out=w_out, w_sc1=w_sc1, w_sc2=w_sc2, w_summ=w_summ, w_q=w_q,
            w_k=w_k, w_v=w_v, w_gate=w_gate, x_bc=x_bc, x_myT=x_myT,
            tok_ids=tok_ids, out_my=out_my))
    nc.compile()
    return nc, dbg


def _body(nc, tc, ES, upto, dbg_out, io):
    alog32 = io["alog32"]
    (x_bf, w_in, conv_w, conv_b, dt_bias, d_in,
     w_out, w_sc1, w_sc2, w_summ, w_q, w_k, w_v, w_gate, x_bc, x_myT,
     tok_ids, out_my) = (
        io["x_bf"], io["w_in"], io["conv_w"], io["conv_b"],
        io["dt_bias"], io["d_in"], io["w_out"],
        io["w_sc1"], io["w_sc2"], io["w_summ"], io["w_q"], io["w_k"], io["w_v"],
        io["w_gate"], io["x_bc"], io["x_myT"], io["tok_ids"], io["out_my"])
    pers = ES.enter_context(tc.tile_pool(name="pers", bufs=1))
    dram = ES.enter_context(tc.tile_pool(name="dram", bufs=1, space="DRAM"))
    # tail-weight tiles reserved up front (stack order); DMAs issued at stage 2
    wts = ES.enter_context(tc.tile_pool(name="wts", bufs=1))
    wo_all = wts.tile([128, 16, 1024], BF, tag="wo_all")
    wg_all = wts.tile([128, 16, 1024], BF, tag="wg_all")
    sc1_all = wts.tile([128, 8, 256], BF, tag="sc1_all")
    wsm_all = wts.tile([128, 8, 64], BF, tag="wsm_all")
    wq_all = wts.tile([128, 8, 64], BF, tag="wq_all")
    wv_all = wts.tile([64, 8, 128], BF, tag="wv_all")
    wkt = wts.tile([64, 64], BF, tag="wkt")
    w2c = wts.tile([128, 2], BF, tag="w2c")

    # ---- shared constants ----
    ident = pers.tile([128, 128], F32, tag="ident")
    with tc.tile_pool(name="tcst", bufs=1) as tcst:
        iod = tcst.tile([128, 128], F32, tag="iod")
        nc.gpsimd.iota(iod[:], pattern=[[1, 128]], base=0, channel_multiplier=-1,
                       allow_small_or_imprecise_dtypes=True)
        nc.vector.tensor_scalar(out=ident[:], in0=iod[:], scalar1=0.0, scalar2=None,
                                op0=OP.is_equal)
    identb = pers.tile([128, 128], BF, tag="identb")
    nc.vector.tensor_copy(identb[:], ident[:])
    identr = pers.tile([128, 128], F32R, tag="identr")
    nc.vector.tensor_copy(identr[:], ident[:])
    epsc = pers.tile([128, 1], F32, tag="epsc")
    nc.vector.memset(epsc[:], EPS)
    ones32 = pers.tile([32, 256], BF, tag="ones32")
    nc.vector.memset(ones32[:], 1.0)
    onescolb = pers.tile([128, 1], BF, tag="onescolb")
    nc.vector.memset(onescolb[:], 1.0)
    onesf = pers.tile([128, 128], F32, tag="onesf")
    nc.vector.memset(onesf[:], 1.0)
    onescol = pers.tile([128, 1], F32R, tag="onescol")
    nc.vector.tensor_copy(onescol[:], onesf[:, 0:1])
    onesb = pers.tile([128, 128], F32R, tag="onesb")
    nc.vector.tensor_copy(onesb[:], onesf[:])

    cwc = pers.tile([128, 4, 4], F32, tag="cwc")
    cbc = pers.tile([128, 4], F32, tag="cbc")
    nc.sync.dma_start(cwc[:], conv_w.rearrange("(a p) k -> p a k", p=128))
    nc.sync.dma_start(cbc[:], conv_b.rearrange("(a p) -> p a", p=128))
    dwk = pers.tile([128, 4, 4, 128], BF, tag="dwk")
    for t in range(4):
        for k in range(4):
            nc.vector.tensor_scalar(out=dwk[:, t, k, :], in0=identb[:],
                                    scalar1=cwc[:, t, k:k + 1], scalar2=None,
                                    op0=OP.mult)

    # ---------------- stage 0+1: rmsnorm stats + in_proj ----------------
    mid_stack = contextlib.ExitStack()
    mid = mid_stack.enter_context(tc.tile_pool(name="mid", bufs=1))
    cv_stack = contextlib.ExitStack()
    cvp = cv_stack.enter_context(tc.tile_pool(name="cvp", bufs=1))
    zT = mid.tile([128, 2, TOK], BF, tag="zT")
    cvin = cvp.tile([128, 2, 2, 1028], BF, tag="cvin")
    dtraw = mid.tile([HC, TOK], F32, tag="dtraw")
    nc.vector.memset(cvin[:, :, :, 0:4], 0.0)

    with tc.tile_pool(name="big", bufs=1) as big, \
         tc.tile_pool(name="t01", bufs=2) as t01, \
         tc.tile_pool(name="psq", bufs=1, space="PSUM") as psq, \
         tc.tile_pool(name="psm", bufs=1, space="PSUM") as psm:
        xTb = big.tile([128, 8, TOK], BF, tag="xTb")
        xbv = x_bf.rearrange("(a p) t -> p a t", p=128)
        for a in range(8):
            nc.sync.dma_start(xTb[:, a, :], xbv[:, a, :])
        wz = big.tile([128, 8, 256], BF, tag="wz")
        wx = big.tile([128, 8, 256], BF, tag="wx")
        wbc = big.tile([128, 8, 256], BF, tag="wbc")
        wdt = big.tile([128, 8, HC], BF, tag="wdt")
        wiv = w_in.rearrange("(a p) c -> p a c", p=128)
        nc.sync.dma_start(wz[:], wiv[:, :, 0:256])
        nc.sync.dma_start(wx[:], wiv[:, :, 256:512])
        nc.sync.dma_start(wbc[:], wiv[:, :, 512:768])
        nc.sync.dma_start(wdt[:], wiv[:, :, 768:772])

        # ---- B/C path, token-sharded: this core's 256 tokens (+4 halo) ----
        xbcb = big.tile([128, 8, 260], BF, tag="xbcb")
        nc.sync.dma_start(xbcb[:], x_bc.rearrange("(a p) t -> p a t", p=128))
        accs_q = [psq.tile([1, 512], F32, name=f"psq{n}", tag=f"psq{n}")
                  for n in range(4)]
        for a in range(8):
            sqb = t01.tile([128, 260], F32R, tag="sqb")
            nc.scalar.activation(sqb[:], xbcb[:, a, :], AF.Square)
            nc.tensor.matmul(accs_q[0][0:1, 0:260], onescol[:], sqb[:],
                             start=(a == 0), stop=(a == 7))
        srtw = big.tile([1, 260], F32, tag="srtw")
        nc.scalar.activation(srtw[:], accs_q[0][0:1, 0:260], AF.Sqrt,
                             bias=epsc[0:1, 0:1], scale=1.0 / D)
        s_bw = big.tile([128, 260], F32, tag="s_bw")
        nc.gpsimd.partition_broadcast(s_bw[:], srtw[:], channels=128)
        nc.vector.reciprocal_approx_fast(out=s_bw[:], in_=s_bw[:])
        cvbc = big.tile([128, 2, 260], BF, tag="cvbc")
        for mb in range(2):
            accb = psm.tile([128, 512], F32, name=f"accb{mb}", tag=f"mmacc{mb}")
            for k in range(8):
                nc.tensor.matmul(accb[:, 0:260], wbc[:, k, 128 * mb:128 * (mb + 1)],
                                 xbcb[:, k, :], start=(k == 0), stop=(k == 7))
            nc.vector.tensor_tensor(cvbc[:, mb, :], accb[:, 0:260], s_bw[:], OP.mult)
        bcm = big.tile([128, 2, 256], BF, tag="bcm")
        for t2 in range(2):
            accc = psm.tile([128, 512], F32, name=f"accc{t2}", tag=f"mmacc{t2 + 2}")
            for k in range(4):
                nc.tensor.matmul(accc[:, 0:256], dwk[:, 2 + t2, k, :],
                                 cvbc[:, t2, k + 1:257 + k], start=(k == 0),
                                 stop=(k == 3))
            nc.scalar.activation(bcm[:, t2, :], accc[:, 0:256], AF.Silu,
                                 bias=cbc[:, 2 + t2:3 + t2])
        agbc_in = dram.tile([256, 256], BF, tag="agbc_in")
        agbc_out = dram.tile([NCORES, 256, 256], BF, tag="agbc_out")
        for t2 in range(2):
            nc.sync.dma_start(agbc_in[128 * t2:128 * (t2 + 1), :], bcm[:, t2, :])
        nc.gpsimd.collective_compute(
            "AllGather", mybir.AluOpType.bypass,
            replica_groups=[list(range(NCORES))],
            ins=[agbc_in.opt()], outs=[agbc_out.opt()],
        )

        # rstd per token: sum of squares via ones-matmul, then Rsqrt
        s_bc = big.tile([128, TOK], F32, tag="s_bc")
        srow = big.tile([1, TOK], F32, tag="srow")
        for a in range(8):
            sq = t01.tile([128, TOK], F32R, tag="sq")
            nc.scalar.activation(sq[:], xTb[:, a, :], AF.Square)
            for n in range(4):
                nc.tensor.matmul(accs_q[n][:], onescol[:], sq[:, 512 * n:512 * (n + 1)],
                                 start=(a == 0), stop=(a == 7))
        srt = big.tile([1, TOK], F32, tag="srt")
        for n in range(4):
            nc.scalar.activation(srt[0:1, 512 * n:512 * (n + 1)], accs_q[n][:],
                                 AF.Sqrt, bias=epsc[0:1, 0:1], scale=1.0 / D)
        # broadcast sqrt first, then wide reciprocal (single-partition
        # reciprocal on [1,2048] costs ~13us; [128,2048] costs ~2us)
        nc.gpsimd.partition_broadcast(s_bc[:], srt[:], channels=128)
        nc.vector.reciprocal_approx_fast(out=s_bc[:], in_=s_bc[:])

        for mi, m in enumerate((6, 2, 3, 0, 1)):
            mm_m = 4 if m == 6 else 128
            accs = [psm.tile([128, 512], F32, name=f"mmacc{n}", tag=f"mmacc{n}")
                    for n in range(4)]
            for k in range(8):
                if m == 6:
                    lhs = wdt[:, k, :]
                else:
                    w_t = (wz, wx, wbc)[m // 2]
                    coff = (m % 2) * 128
                    lhs = w_t[:, k, coff:coff + 128]
                for n in range(4):
                    n0 = 512 * n
                    nc.tensor.matmul(accs[n][0:mm_m, :], lhs, xTb[:, k, n0:n0 + 512],
                                     start=(k == 0), stop=(k == 7))
            for n in range(4):
                n0 = 512 * n
                sb = s_bc[0:mm_m, n0:n0 + 512]
                if m < 2:
                    dst = zT[:, m, n0:n0 + 512]
                elif m < 6:
                    b = n0 // 1024
                    dst = cvin[:, m - 2, b, 4 + (n0 % 1024):4 + (n0 % 1024) + 512]
                else:
                    dst = dtraw[:, n0:n0 + 512]
                nc.vector.tensor_tensor(dst, accs[n][0:mm_m, :], sb, OP.mult)

    zx_dbg = dbg_out("zx_dbg", [768, TOK], BF)
    if zx_dbg is not None:
        nc.sync.dma_start(zx_dbg[0:256, :].rearrange("(a p) t -> p a t", p=128), zT[:])
        for mi in range(2):
            for b in range(2):
                nc.sync.dma_start(
                    zx_dbg[256 + 128 * mi:384 + 128 * mi, 1024 * b:1024 * (b + 1)],
                    cvin[:, mi, b, 4:1028])
    dtr_dbg = dbg_out("dtr_dbg", [HC, TOK], F32)
    if dtr_dbg is not None:
        nc.sync.dma_start(dtr_dbg[:], dtraw[:])
    if upto < 2:
        cv_stack.close(); mid_stack.close()
        return

    # ---------------- stage 2: conv+silu, dt/dA/cs ----------------
    ssd_stack = contextlib.ExitStack()
    ssd = ssd_stack.enter_context(tc.tile_pool(name="ssd", bufs=1))
    xsT = ssd.tile([128, 2, TOK], BF, tag="xsT")
    bT = ssd.tile([128, TOK], BF, tag="bT")
    cT = ssd.tile([128, TOK], BF, tag="cT")
    dtt = ssd.tile([HC, TOK], F32, tag="dtt")
    yT = ssd.tile([128, 2, TOK], BF, tag="yT")
    g = ssd.tile([128, 2, TOK], BF, tag="g")

    # silu(z) applied once in place (z is only ever consumed through silu);
    # avoids per-chunk Silu<->Exp activation-table thrash inside the SSD loop
    for cb in range(2):
        nc.scalar.activation(zT[:, cb, :], zT[:, cb, :], AF.Silu)

    # x-part depthwise conv (B/C arrive via the early AllGather)
    nc.sync.dma_start(bT[:].rearrange("c (j t) -> c j t", j=8),
                      agbc_out[:, 0:128, :].rearrange("j c t -> c j t"))
    nc.sync.dma_start(cT[:].rearrange("c (j t) -> c j t", j=8),
                      agbc_out[:, 128:256, :].rearrange("j c t -> c j t"))
    with tc.tile_pool(name="cps", bufs=2, space="PSUM") as cps:
        for t in range(2):
            for b in range(2):
                for hv in range(2):
                    acc = cps.tile([128, 512], F32, tag="cacc")
                    o = 512 * hv
                    for k in range(4):
                        nc.tensor.matmul(acc[:], dwk[:, t, k, :],
                                         cvin[:, t, b, o + k + 1:o + 513 + k],
                                         start=(k == 0), stop=(k == 3))
                    bsl = slice(1024 * b + o, 1024 * b + o + 512)
                    nc.scalar.activation(xsT[:, t, bsl], acc[:], AF.Silu,
                                         bias=cbc[:, t:t + 1])

    dtb_c = pers.tile([HC, 1], F32, tag="dtb_c")
    nc.sync.dma_start(dtb_c[:], dt_bias[:])
    alog32_c = pers.tile([32, 1], F32, tag="alog32_c")
    nc.sync.dma_start(alog32_c[:], alog32[:])
    # softplus(x+b) = -ln(sigmoid(-(x+b)))
    dtbn = pers.tile([HC, 1], F32, tag="dtbn")
    nc.vector.tensor_scalar(out=dtbn[:], in0=dtb_c[:], scalar1=-1.0, scalar2=None,
                            op0=OP.mult)
    nc.scalar.activation(dtt[:], dtraw[:], AF.Sigmoid, scale=-1.0, bias=dtbn[:, 0:1])
    nc.scalar.activation(dtt[:], dtt[:], AF.Ln)
    nc.vector.tensor_scalar(out=dtt[:], in0=dtt[:], scalar1=-1.0, scalar2=None,
                            op0=OP.mult)

    dt_dbg = dbg_out("dt_dbg", [HC, TOK], F32)
    if dt_dbg is not None:
        nc.sync.dma_start(dt_dbg[:], dtt[:])
    xbc_dbg = dbg_out("xbc_dbg", [512, TOK], BF)
    if xbc_dbg is not None:
        nc.sync.dma_start(xbc_dbg[0:256, :].rearrange("(a p) t -> p a t", p=128), xsT[:])
        nc.sync.dma_start(xbc_dbg[256:384, :], bT[:])
        nc.sync.dma_start(xbc_dbg[384:512, :], cT[:])
    if upto < 3:
        ssd_stack.close(); cv_stack.close(); mid_stack.close()
        return

    # ---------------- stage 3: SSD ----------------
    # dAr (32 rows = h*8 + b*4 + c, 256): DMA from dtt then scale by -exp(A_log)
    dAr = ssd.tile([32, L], F32, tag="dAr")
    csr = ssd.tile([32, L], F32R, tag="csr")
    expcs = ssd.tile([32, L], F32R, tag="expcs")
    expa32 = ssd.tile([32, 1], F32, tag="expa32")
    nc.scalar.activation(expa32[:], alog32_c[:], AF.Exp)
    for bc in range(8):
        b, c = bc // 4, bc % 4
        nc.sync.dma_start(dAr[bc:bc + 25:8, :],
                          dtt[:, 1024 * b + 256 * c:1024 * b + 256 * (c + 1)])
    nc.vector.tensor_scalar(out=dAr[:], in0=dAr[:], scalar1=expa32[:, 0:1], scalar2=-1.0,
                            op0=OP.mult, op1=OP.mult)
    nc.vector.tensor_tensor_scan(csr[:], dAr[:], dAr[:], 0.0, OP.add, OP.bypass)
    nc.scalar.activation(expcs[:], csr[:], AF.Exp)
    decay_r = ssd.tile([32, L], F32, tag="decay_r")
    nc.scalar.activation(decay_r[:], csr[:], AF.Exp, scale=-1.0, bias=csr[:, L - 1:L])

    # prefetch tail weights (overlaps SSD compute; behind the staging DMAs)
    nc.sync.dma_start(wo_all[:], w_out.rearrange("(a p) m -> p a m", p=128))
    nc.sync.dma_start(wg_all[:], w_gate.rearrange("(a p) m -> p a m", p=128))
    nc.sync.dma_start(sc1_all[:], w_sc1.rearrange("(a p) m -> p a m", p=128))
    nc.sync.dma_start(wsm_all[:], w_summ.rearrange("(a p) m -> p a m", p=128))
    nc.sync.dma_start(wq_all[:], w_q.rearrange("(a p) m -> p a m", p=128))
    nc.sync.dma_start(wv_all[:], w_v.rearrange("s (a m) -> s a m", m=128))
    nc.sync.dma_start(wkt[:], w_k[:])
    nc.sync.dma_start(w2c[:], w_sc2.rearrange("(a p) o -> p (a o)", p=128))

    # s-major columns: dec as (128, 2st, 32r) with r = h*8+bc; dt per (b,c)
    dec_col = ssd.tile([128, 2, 32], F32, tag="dec_col")
    dt_col = ssd.tile([128, 2, 8, HC], F32, tag="dt_col")
    dtdec_col = ssd.tile([128, 2, 8, HC], F32, tag="dtdec_col")
    cs_colT = ssd.tile([128, 2, 32], F32, tag="cs_colT")
    with tc.tile_pool(name="psmt", bufs=2, space="PSUM") as psmt:
        for st in range(2):
            pt = psmt.tile([128, 32], F32, tag="mt32")
            nc.tensor.transpose(pt[:], decay_r[:, 128 * st:128 * (st + 1)],
                                ident[0:32, 0:32])
            nc.scalar.activation(dec_col[:, st, :], pt[:], AF.Copy)
            ptc = psmt.tile([128, 32], F32, tag="mt32c")
            nc.tensor.transpose(ptc[:].bitcast(F32R), csr[:, 128 * st:128 * (st + 1)],
                                identr[0:32, 0:32])
            nc.scalar.activation(cs_colT[:, st, :], ptc[:], AF.Copy)
            for bc in range(8):
                b, c = bc // 4, bc % 4
                pt2 = psmt.tile([128, HC], F32, tag="mt")
                t0 = 1024 * b + 256 * c + 128 * st
                nc.tensor.transpose(pt2[:], dtt[:, t0:t0 + 128], ident[0:HC, 0:HC])
                nc.scalar.activation(dt_col[:, st, bc, :], pt2[:], AF.Copy)
            for bc in range(8):
                nc.vector.tensor_tensor(dtdec_col[:, st, bc, :], dt_col[:, st, bc, :],
                                        dec_col[:, st, bc:bc + 25:8], OP.mult)

    # exp(cs_last) per row, broadcast to all partitions: etb [128, 32]
    etb = ssd.tile([128, 32], F32, tag="etb")
    with tc.tile_pool(name="pset", bufs=1, space="PSUM") as pset, \
         tc.tile_pool(name="tet", bufs=1) as tet:
        ptl = pset.tile([1, 32], F32, tag="ptl")
        nc.tensor.transpose(ptl[:].bitcast(F32R), csr[:, L - 1:L], identr[0:32, 0:32])
        etrow = tet.tile([1, 32], F32, tag="etrow")
        nc.scalar.activation(etrow[:], ptl[:], AF.Exp)
        nc.gpsimd.partition_broadcast(etb[:], etrow[:], channels=128)

    # mask bias: mb[p, st, l] = 0 if l >= 128*st+p else -1e30
    maskb = ssd.tile([128, 2, L], F32, tag="maskb")
    with tc.tile_pool(name="tio2", bufs=2) as tio2:
        for st in range(2):
            iol2 = tio2.tile([128, L], F32, tag="iol2")
            nc.gpsimd.iota(iol2[:], pattern=[[1, L]], base=-128 * st,
                           channel_multiplier=-1, allow_small_or_imprecise_dtypes=True)
            nc.vector.tensor_scalar(out=maskb[:, st, :], in0=iol2[:], scalar1=0.0,
                                    scalar2=-1e30, op0=OP.is_lt, op1=OP.mult)

    # D per pair-of-heads column (pre-broadcast on host)
    dDc = pers.tile([128, 2], F32, tag="dDc")
    nc.sync.dma_start(dDc[:], d_in[:])


    a2a_in = dram.tile([NCORES, CH, MYTOK], BF, tag="a2a_in")
    a2a_out = dram.tile([NCORES, CH, MYTOK], BF, tag="a2a_out")

    Rr = ssd.tile([128, 2, CH], BF, tag="Rr")
    with tc.tile_pool(name="psT", bufs=1, space="PSUM") as psT, \
         tc.tile_pool(name="psS2", bufs=1, space="PSUM") as psS2, \
         tc.tile_pool(name="psG", bufs=1, space="PSUM") as psG, \
         tc.tile_pool(name="psY", bufs=2, space="PSUM") as psY, \
         tc.tile_pool(name="psB", bufs=1, space="PSUM") as psB, \
         tc.tile_pool(name="tT", bufs=2) as tT, \
         tc.tile_pool(name="tG", bufs=3) as tG, \
         tc.tile_pool(name="tM", bufs=3) as tM:
        for b in range(2):
            for c in range(4):
                bc = b * 4 + c
                t0 = 1024 * b + 256 * c
                # csr/expcs rows for this chunk staged at partitions {0,64}
                # (PE base-partition alignment); scalar-queue DMAs so they
                # don't queue behind the big weight prefetch
                cs4a = tT.tile([128, L], F32R, tag="cs4a")
                cs4b = tT.tile([128, L], F32R, tag="cs4b")
                nc.scalar.dma_start(cs4a[0:65:64, :], csr[bc:bc + 9:8, :])
                nc.scalar.dma_start(cs4b[0:65:64, :], csr[bc + 16:bc + 25:8, :])
                if c > 0:
                    ec4a = tT.tile([128, L], F32R, tag="ec4a")
                    ec4b = tT.tile([128, L], F32R, tag="ec4b")
                    nc.scalar.dma_start(ec4a[0:65:64, :], expcs[bc:bc + 9:8, :])
                    nc.scalar.dma_start(ec4b[0:65:64, :], expcs[bc + 16:bc + 25:8, :])
                # per-chunk transposes: xd/xdd (s-major), bS
                xd = tT.tile([128, 2, CH], BF, tag="xd")
                xdd = tT.tile([128, 2, CH], BF, tag="xdd")
                bS = tT.tile([128, 2, NSTATE], BF, tag="bS")
                for st in range(2):
                    ts0 = t0 + 128 * st
                    for cb in range(2):
                        pt = psT.tile([128, 128], BF, tag="xdt")
                        nc.tensor.transpose(pt[:], xsT[:, cb, ts0:ts0 + 128], identb[:])
                        for hh in range(2):
                            h = 2 * cb + hh
                            nc.vector.tensor_scalar(
                                out=xd[:, st, 64 * h:64 * (h + 1)],
                                in0=pt[:, 64 * hh:64 * (hh + 1)],
                                scalar1=dt_col[:, st, bc, h:h + 1], scalar2=None,
                                op0=OP.mult)
                            nc.vector.tensor_scalar(
                                out=xdd[:, st, 64 * h:64 * (h + 1)],
                                in0=pt[:, 64 * hh:64 * (hh + 1)],
                                scalar1=dtdec_col[:, st, bc, h:h + 1], scalar2=None,
                                op0=OP.mult)
                    ptb = psT.tile([128, 128], BF, tag="bdt")
                    nc.tensor.transpose(ptb[:], bT[:, ts0:ts0 + 128], identb[:])
                    nc.scalar.activation(bS[:, st, :], ptb[:], AF.Copy)
                # states S (n, (h,p))
                sps = psS2.tile([128, CH], F32, tag="sps")
                for st in range(2):
                    nc.tensor.matmul(sps[:], bS[:, st, :], xdd[:, st, :],
                                     start=(st == 0), stop=(st == 1))
                S = tT.tile([128, CH], F32, tag="S")
                nc.scalar.activation(S[:], sps[:], AF.Copy)
                # Gt (s,l) shared across heads
                gts = []
                for st in range(2):
                    pg = psG.tile([128, L], F32, tag="pg")
                    nc.tensor.matmul(pg[:], bT[:, t0 + 128 * st:t0 + 128 * (st + 1)],
                                     cT[:, t0:t0 + 256], start=True, stop=True)
                    gt = tG.tile([128, L], BF, tag="gt")
                    nc.scalar.activation(gt[:], pg[:], AF.Copy)
                    gts.append(gt)
                for h in range(HC):
                    cb, hh = h // 2, h % 2
                    r = h * 8 + bc
                    hp = 64 * (h % 2)
                    cs4 = cs4a if h < 2 else cs4b
                    # broadcast csr row r to all partitions via rank-1 matmul
                    csp = psB.tile([128, L], F32, tag="csp")
                    nc.tensor.matmul(csp[:], onesb[hp:hp + 1, :], cs4[hp:hp + 1, :],
                                     start=True, stop=True)
                    ce = None
                    if c > 0:
                        ec4 = ec4a if h < 2 else ec4b
                        ecp = psB.tile([128, L], F32, tag="ecp")
                        nc.tensor.matmul(ecp[:], onesb[hp:hp + 1, :], ec4[hp:hp + 1, :],
                                         start=True, stop=True)
                        ce = tM.tile([128, L], BF, tag="ce")
                        nc.vector.tensor_tensor(ce[:], cT[:, t0:t0 + 256], ecp[:],
                                                OP.mult)
                    psl = slice(64 * hh, 64 * (hh + 1))
                    ypb = psY.tile([64, L], F32, tag="ypb")
                    ltp2 = tM.tile([128, 2, L], F32, tag="ltp2")
                    for st in range(2):
                        nc.vector.scalar_tensor_tensor(
                            out=ltp2[:, st, :], in0=csp[:],
                            scalar=cs_colT[:, st, r:r + 1],
                            in1=maskb[:, st, :], op0=OP.subtract, op1=OP.add)
                    lt2 = tM.tile([128, 2, L], BF, tag="lt2")
                    nc.scalar.activation(lt2[:], ltp2[:], AF.Exp)
                    for st in range(2):
                        ms = tM.tile([128, L], BF, tag="ms")
                        nc.gpsimd.tensor_tensor(ms[:], lt2[:, st, :], gts[st][:],
                                                OP.mult)
                        nc.tensor.matmul(ypb[:], xd[:, st, 64 * h:64 * (h + 1)],
                                         ms[:], start=(st == 0),
                                         stop=(st == 1 and c == 0))
                    if c > 0:
                        nc.tensor.matmul(ypb[:], Rr[:, b, 64 * h:64 * (h + 1)],
                                         ce[:], start=False, stop=True)
                    nc.vector.scalar_tensor_tensor(
                        out=yT[psl, cb, t0:t0 + 256], in0=xsT[psl, cb, t0:t0 + 256],
                        scalar=dDc[psl, cb:cb + 1], in1=ypb[0:64, :],
                        op0=OP.mult, op1=OP.add)
                # R update for next chunk
                if c == 0:
                    nc.scalar.activation(Rr[:, b, :], S[:], AF.Copy)
                elif c < 3:
                    for h in range(HC):
                        r = h * 8 + bc
                        nc.vector.scalar_tensor_tensor(
                            out=Rr[:, b, 64 * h:64 * (h + 1)],
                            in0=Rr[:, b, 64 * h:64 * (h + 1)],
                            scalar=etb[:, r:r + 1],
                            in1=S[:, 64 * h:64 * (h + 1)], op0=OP.mult, op1=OP.add)
                # gated z*silu for this chunk + A2A staging (dest core j == bc)
                for cb in range(2):
                    nc.gpsimd.tensor_tensor(g[:, cb, t0:t0 + 256],
                                            yT[:, cb, t0:t0 + 256],
                                            zT[:, cb, t0:t0 + 256], OP.mult)
                    nc.sync.dma_start(a2a_in[bc, 128 * cb:128 * (cb + 1), :],
                                      g[:, cb, t0:t0 + 256])

    y_dbg = dbg_out("y_dbg", [CH, TOK], BF)
    if y_dbg is not None:
        nc.sync.dma_start(y_dbg.rearrange("(a p) t -> p a t", p=128), yT[:])
    g_dbg = dbg_out("g_dbg", [CH, TOK], BF)
    if g_dbg is not None:
        nc.sync.dma_start(g_dbg.rearrange("(a p) t -> p a t", p=128), g[:])
    if upto < 5:
        ssd_stack.close(); cv_stack.close(); mid_stack.close()
        return

    # ---------------- stage 5: AllToAll ----------------
    ssd_stack.close()
    cv_stack.close()
    mid_stack.close()
    nc.gpsimd.collective_compute(
        "AllToAll", mybir.AluOpType.bypass,
        replica_groups=[list(range(NCORES))],
        ins=[a2a_in.opt()], outs=[a2a_out.opt()],
    )
    st6 = ES.enter_context(tc.tile_pool(name="st6", bufs=1))
    g2 = st6.tile([128, 16, MYTOK], BF, tag="g2")
    nc.sync.dma_start(g2[:], a2a_out.rearrange("j (cb p) t -> p (j cb) t", p=128))

    # ---------------- stage 5b: gated RMSNorm (token-local) ----------------
    # ssm_norm_w is folded into w_out on the host, so yn = g2 * rstd only.
    yn = st6.tile([128, 16, MYTOK], BF, tag="yn")
    rstd_bc = st6.tile([128, MYTOK], F32, tag="rstd_bc")
    with tc.tile_pool(name="tn", bufs=3) as tn, \
         tc.tile_pool(name="psn", bufs=1, space="PSUM") as psn:
        ssps = psn.tile([1, MYTOK], F32, tag="ssps")
        for ip in range(8):
            gsq = tn.tile([128, 2, MYTOK], BF, tag="gsq")
            nc.scalar.activation(gsq[:], g2[:, 2 * ip:2 * ip + 2, :], AF.Square)
            for j in range(2):
                nc.tensor.matmul(ssps[:], onescolb[:], gsq[:, j, :],
                                 start=(ip == 0 and j == 0),
                                 stop=(ip == 7 and j == 1))
        rstd_s = tn.tile([1, MYTOK], F32, tag="rstd_s")
        nc.scalar.activation(rstd_s[:], ssps[:], AF.Sqrt, bias=epsc[0:1, 0:1],
                             scale=1.0 / DI)
        nc.gpsimd.partition_broadcast(rstd_bc[:], rstd_s[:], channels=128)
        nc.vector.reciprocal_approx_fast(out=rstd_bc[:], in_=rstd_bc[:])
    for i in range(16):
        nc.vector.tensor_tensor(yn[:, i, :], g2[:, i, :], rstd_bc[:], OP.mult)
    yn_dbg = dbg_out("yn_dbg", [DI, MYTOK], BF)
    if yn_dbg is not None:
        nc.sync.dma_start(yn_dbg.rearrange("(a p) t -> p a t", p=128), yn[:])
    if upto < 6:
        return

    # ---------------- stage 6a: out_proj, scorer, summaries, q ----------------
    y2 = st6.tile([128, 8, MYTOK], BF, tag="y2")
    with tc.tile_pool(name="ps6", bufs=2, space="PSUM") as ps6:
        for m in range(8):
            acc = ps6.tile([128, MYTOK], F32, tag="oacc")
            for k in range(16):
                nc.tensor.matmul(acc[:], wo_all[:, k, 128 * m:128 * (m + 1)], yn[:, k, :],
                                 start=(k == 0), stop=(k == 15))
            if m % 2 == 0:
                nc.vector.tensor_copy(y2[:, m, :], acc[:])
            else:
                nc.scalar.activation(y2[:, m, :], acc[:], AF.Copy)

        # scorer
        rl1 = st6.tile([128, 2, MYTOK], BF, tag="rl1")
        for m in range(2):
            acc = ps6.tile([128, MYTOK], F32, tag="oacc")
            for k in range(8):
                nc.tensor.matmul(acc[:], sc1_all[:, k, 128 * m:128 * (m + 1)], y2[:, k, :],
                                 start=(k == 0), stop=(k == 7))
            nc.scalar.activation(rl1[:, m, :], acc[:], AF.Relu)
        u_row = st6.tile([1, MYTOK], F32, tag="u_row")
        ups = ps6.tile([1, MYTOK], F32, tag="ups")
        for m in range(2):
            nc.tensor.matmul(ups[:], w2c[:, m:m + 1], rl1[:, m, :], start=(m == 0),
                             stop=(m == 1))
        nc.vector.tensor_copy(u_row[:], ups[:])

        # summaries + q (64-col projections of y2)
        summT = st6.tile([64, MYTOK], BF, tag="summT")
        qT = st6.tile([64, MYTOK], BF, tag="qT")
        for (wv_t, dst) in ((wsm_all, summT), (wq_all, qT)):
            acc = ps6.tile([64, MYTOK], F32, tag="sacc6")
            for k in range(8):
                nc.tensor.matmul(acc[:], wv_t[:, k, :], y2[:, k, :], start=(k == 0),
                                 stop=(k == 7))
            nc.scalar.activation(dst[:], acc[:], AF.Copy)

        # summaries token-major (for allgather), as f32
        stm = st6.tile([128, 2, 64], F32, tag="stm")
        for st in range(2):
            pt = ps6.tile([128, 64], BF, tag="stp")
            nc.tensor.transpose(pt[:], summT[:, 128 * st:128 * (st + 1)],
                                identb[0:64, 0:64])
            nc.scalar.activation(stm[:, st, :], pt[:], AF.Copy)

    # gate phase 1: y2 half (overlaps the pool-selection collectives)
    gy2 = st6.tile([128, 8, MYTOK], F32, tag="gy2")
    with tc.tile_pool(name="psg1", bufs=2, space="PSUM") as psg1:
        for m in range(8):
            acc = psg1.tile([128, MYTOK], F32, tag="g1acc")
            for k in range(8):
                nc.tensor.matmul(acc[:], wg_all[:, k, 128 * m:128 * (m + 1)], y2[:, k, :],
                                 start=(k == 0), stop=(k == 7))
            nc.scalar.activation(gy2[:, m, :], acc[:], AF.Copy)

    y2_dbg = dbg_out("y2_dbg", [D, MYTOK], BF)
    if y2_dbg is not None:
        nc.sync.dma_start(y2_dbg.rearrange("(a p) t -> p a t", p=128), y2[:])
    u_dbg = dbg_out("u_dbg", [1, MYTOK], F32)
    if u_dbg is not None:
        nc.sync.dma_start(u_dbg[:], u_row[:])
    if upto < 7:
        return

    # ---------------- stage 6b: allgather u+summaries, ranks, members ----------------
    ag1_in = dram.tile([MYTOK, 65], F32, tag="ag1_in")
    ag1_out = dram.tile([NCORES, MYTOK, 65], F32, tag="ag1_out")
    nc.sync.dma_start(ag1_in[:, 0:1].rearrange("t o -> o t"), u_row[:])
    for st in range(2):
        nc.sync.dma_start(ag1_in[128 * st:128 * (st + 1), 1:65], stm[:, st, :])
    nc.gpsimd.collective_compute(
        "AllGather", mybir.AluOpType.bypass,
        replica_groups=[list(range(NCORES))],
        ins=[ag1_in.opt()], outs=[ag1_out.opt()],
    )
    u_all = st6.tile([1, TOK], F32, tag="u_all")
    nc.sync.dma_start(u_all[:], ag1_out[:, :, 0:1].rearrange("j t o -> o (j t)"))
    summ_all = st6.tile([128, 16, 64], F32, tag="summ_all")
    summ_allr = st6.tile([128, 16, 64], BF, tag="summ_allr")
    nc.sync.dma_start(summ_all[:],
                      ag1_out[:, :, 1:65].rearrange("j (st p) c -> p (j st) c", p=128))

    # ranks for my 256 tokens vs all 2048
    u_bc = st6.tile([128, TOK], F32, tag="u_bc")
    nc.gpsimd.partition_broadcast(u_bc[:], u_all[:], channels=128)
    iota_all = st6.tile([128, TOK], F32, tag="iota_all")
    nc.gpsimd.iota(iota_all[:], pattern=[[1, TOK]], base=0, channel_multiplier=0,
                   allow_small_or_imprecise_dtypes=True)
    u_col = st6.tile([128, 2], F32, tag="u_col")
    with tc.tile_pool(name="psu", bufs=2, space="PSUM") as psu:
        for st in range(2):
            pu = psu.tile([128, 1], F32, tag="pu")
            nc.tensor.transpose(pu[:], u_row[0:1, 128 * st:128 * (st + 1)],
                                ident[0:1, 0:1])
            nc.vector.tensor_copy(u_col[:, st:st + 1], pu[:])
    tid_col = st6.tile([128, 2], F32, tag="tid_col")
    nc.sync.dma_start(tid_col[:], tok_ids.rearrange("(a p) o -> p (a o)", p=128))
    member = st6.tile([128, 2], F32, tag="member")
    with tc.tile_pool(name="trk", bufs=1) as trk:
        for st in range(2):
            junk = trk.tile([128, TOK], F32, tag="junk")
            rgt = trk.tile([128, 1], F32, tag="rgt")
            nc.vector.tensor_scalar(out=junk[:], in0=u_bc[:], scalar1=u_col[:, st:st + 1],
                                    scalar2=0.0, op0=OP.is_gt, op1=OP.add,
                                    accum_out=rgt[:])
            eqm = trk.tile([128, TOK], F32, tag="eqm")
            nc.vector.tensor_scalar(out=eqm[:], in0=u_bc[:], scalar1=u_col[:, st:st + 1],
                                    scalar2=None, op0=OP.is_equal)
            junk2 = trk.tile([128, TOK], F32, tag="junk2")
            req = trk.tile([128, 1], F32, tag="req")
            nc.vector.scalar_tensor_tensor(out=junk2[:], in0=iota_all[:],
                                           scalar=tid_col[:, st:st + 1], in1=eqm[:],
                                           op0=OP.is_lt, op1=OP.mult, accum_out=req[:])
            rank = trk.tile([128, 1], F32, tag="rank")
            nc.vector.tensor_tensor(rank[:], rgt[:], req[:], OP.add)
            rlt = trk.tile([128, 1], F32, tag="rlt")
            nc.vector.tensor_scalar(out=rlt[:], in0=rank[:], scalar1=float(KRANK),
                                    scalar2=None, op0=OP.is_lt)
            vld = trk.tile([128, 1], F32, tag="vld")
            nc.vector.tensor_scalar(out=vld[:], in0=u_col[:, st:st + 1], scalar1=0.0,
                                    scalar2=None, op0=OP.is_gt)
            nc.vector.tensor_tensor(member[:, st:st + 1], rlt[:], vld[:], OP.mult)

    ag2_in = dram.tile([MYTOK, 1], F32, tag="ag2_in")
    ag2_out = dram.tile([NCORES, MYTOK, 1], F32, tag="ag2_out")
    for st in range(2):
        nc.sync.dma_start(ag2_in[128 * st:128 * (st + 1), :], member[:, st:st + 1])
    nc.gpsimd.collective_compute(
        "AllGather", mybir.AluOpType.bypass,
        replica_groups=[list(range(NCORES))],
        ins=[ag2_in.opt()], outs=[ag2_out.opt()],
    )

    # ---------------- stage 6c: retrieval (logits/exp overlap AG2) ----------------
    summT_all = st6.tile([64, TOK], BF, tag="summT_all")
    esm = st6.tile([128, 16, MYTOK], BF, tag="esm")
    retrT = st6.tile([128, 8, MYTOK], BF, tag="retrT")
    # cond from u_all (overlaps AG2 as well)
    cond_col = st6.tile([128, 1], F32, tag="cond_col")
    with tc.tile_pool(name="tcd", bufs=1) as tcd:
        sgj = tcd.tile([1, TOK], F32, tag="sgj")
        sgs = tcd.tile([1, 1], F32, tag="sgs")
        nc.scalar.activation(sgj[:], u_all[:], AF.Sigmoid, accum_out=sgs[:])
        vj = tcd.tile([1, TOK], F32, tag="vj")
        vs = tcd.tile([1, 1], F32, tag="vs")
        nc.vector.tensor_scalar(out=vj[:], in0=u_all[:], scalar1=0.0, scalar2=0.0,
                                op0=OP.is_gt, op1=OP.add, accum_out=vs[:])
        c1 = tcd.tile([1, 1], F32, tag="c1")
        nc.vector.tensor_scalar(out=c1[:], in0=sgs[:], scalar1=float(TAU2 * TOK),
                                scalar2=None, op0=OP.is_gt)
        c2 = tcd.tile([1, 1], F32, tag="c2")
        nc.vector.tensor_scalar(out=c2[:], in0=vs[:], scalar1=0.0, scalar2=None,
                                op0=OP.is_gt)
        cnd = tcd.tile([1, 1], F32, tag="cnd")
        nc.vector.tensor_tensor(cnd[:], c1[:], c2[:], OP.mult)
        nc.gpsimd.partition_broadcast(cond_col[:], cnd[:], channels=128)

    xres = st6.tile([128, 8, MYTOK], F32, tag="xres")
    nc.sync.dma_start(xres[:], x_myT.rearrange("(a p) t -> p a t", p=128))

    with tc.tile_pool(name="tr6", bufs=4) as tr6, \
         tc.tile_pool(name="psr6", bufs=1, space="PSUM") as psr6, \
         tc.tile_pool(name="psl6", bufs=2, space="PSUM") as psl6, \
         tc.tile_pool(name="psrp", bufs=1, space="PSUM") as psrp, \
         tc.tile_pool(name="psq6", bufs=1, space="PSUM") as psq6:
        for i in range(16):
            pt = psq6.tile([64, 128], F32, tag="satp")
            nc.tensor.transpose(pt[:], summ_all[:, i, :], ident[:])
            nc.scalar.activation(summT_all[:, 128 * i:128 * (i + 1)], pt[:], AF.Copy)
        wktp = psr6.tile([64, 64], BF, tag="wktp")
        nc.tensor.transpose(wktp[:], wkt[:], identb[0:64, 0:64])
        kwT = tr6.tile([64, 64], BF, tag="kwT")
        nc.scalar.activation(kwT[:], wktp[:], AF.Copy)
        kqp = psr6.tile([64, MYTOK], F32, tag="kqp")
        nc.tensor.matmul(kqp[:], kwT[:], qT[:], start=True, stop=True)
        kq = tr6.tile([64, MYTOK], BF, tag="kq")
        nc.vector.tensor_scalar(out=kq[:], in0=kqp[:], scalar1=0.25, scalar2=None,
                                op0=OP.mult)
        # unmasked exp(logits) — mask applied multiplicatively after AG2
        for i in range(16):
            lp = psl6.tile([128, MYTOK], F32, tag="lp")
            nc.tensor.matmul(lp[:], summT_all[:, 128 * i:128 * (i + 1)], kq[:],
                             start=True, stop=True)
            nc.scalar.activation(esm[:, i, :], lp[:], AF.Exp)
        mask_col = st6.tile([128, 16], F32, tag="mask_col")
        nc.sync.dma_start(mask_col[:], ag2_out.rearrange("j (a p) o -> p (j a o)", p=128))
        mem_dbg = dbg_out("mem_dbg", [1, TOK], F32)
        if mem_dbg is not None:
            nc.sync.dma_start(mem_dbg[:], ag2_out.rearrange("j t o -> o (j t)"))
        # masked bf16 summaries (mask folded into the f32->bf16 cast) and
        # softmax denominator as a mask-vector matmul on the tensor engine
        maskb16 = tr6.tile([128, 16], BF, tag="maskb16")
        nc.vector.tensor_scalar(out=maskb16[:], in0=mask_col[:], scalar1=1.0,
                                scalar2=None, op0=OP.mult)
        for i in range(16):
            nc.scalar.activation(summ_allr[:, i, :], summ_all[:, i, :], AF.Copy,
                                 scale=mask_col[:, i:i + 1])
        denps = psr6.tile([1, MYTOK], F32, tag="denps")
        for i in range(16):
            nc.tensor.matmul(denps[:], maskb16[:, i:i + 1], esm[:, i, :],
                             start=(i == 0), stop=(i == 15))
        den_row = tr6.tile([1, MYTOK], F32, tag="den_row")
        nc.scalar.activation(den_row[:], denps[:], AF.Copy)
        rden_bc = tr6.tile([64, MYTOK], F32, tag="rden_bc")
        nc.gpsimd.partition_broadcast(rden_bc[:], den_row[:], channels=64)
        nc.vector.reciprocal_approx_fast(out=rden_bc[:], in_=rden_bc[:])
        tmpp = psr6.tile([64, MYTOK], F32, tag="tmpp")
        for i in range(16):
            nc.tensor.matmul(tmpp[:], summ_allr[:, i, :], esm[:, i, :], start=(i == 0),
                             stop=(i == 15))
        tmps = tr6.tile([64, MYTOK], BF, tag="tmps")
        nc.vector.tensor_tensor(tmps[:], tmpp[:], rden_bc[:], OP.mult)
        for m in range(8):
            rp = psrp.tile([128, MYTOK], F32, tag="rp")
            nc.tensor.matmul(rp[:], wv_all[:, m, :], tmps[:], start=True, stop=True)
            if m % 2 == 0:
                nc.vector.tensor_copy(retrT[:, m, :], rp[:])
            else:
                nc.scalar.activation(retrT[:, m, :], rp[:], AF.Copy)

    retr_dbg = dbg_out("retr_dbg", [D, MYTOK], BF)
    if retr_dbg is not None:
        nc.sync.dma_start(retr_dbg.rearrange("(a p) t -> p a t", p=128), retrT[:])
    if upto < 9:
        return

    # ---------------- stage 6d: gate, final ----------------
    with tc.tile_pool(name="psg6", bufs=3, space="PSUM") as psg6, \
         tc.tile_pool(name="tf6", bufs=3) as tf6:
        for m in range(8):
            acc = psg6.tile([128, MYTOK], F32, tag="gacc")
            for k in range(8, 16):
                nc.tensor.matmul(acc[:], wg_all[:, k, 128 * m:128 * (m + 1)],
                                 retrT[:, k - 8, :], start=(k == 8), stop=(k == 15))
            gl = tf6.tile([128, MYTOK], F32, tag="gl")
            nc.vector.tensor_tensor(gl[:], acc[:], gy2[:, m, :], OP.add)
            gsb = tf6.tile([128, MYTOK], F32, tag="gsb")
            nc.scalar.activation(gsb[:], gl[:], AF.Sigmoid)
            t1 = tf6.tile([128, MYTOK], F32, tag="t1")
            nc.vector.tensor_tensor(t1[:], gsb[:], retrT[:, m, :], OP.mult)
            t2 = tf6.tile([128, MYTOK], F32, tag="t2")
            nc.vector.scalar_tensor_tensor(out=t2[:], in0=t1[:], scalar=cond_col[:, 0:1],
                                           in1=y2[:, m, :], op0=OP.mult, op1=OP.add)
            fin = tf6.tile([128, MYTOK], F32, tag="fin")
            nc.vector.tensor_tensor(fin[:], t2[:], xres[:, m, :], OP.add)
            nc.sync.dma_start(out_my[128 * m:128 * (m + 1), :], fin[:])


# ---- host-side sharding ----


def _bc_window(x_padT, k):
    # conv halo for this core's 256 tokens; zeroed at batch boundaries
    # (reference pads the causal conv per batch: batch 1 starts at token 1024)
    w = x_padT[:, 256 * k:256 * k + 260].copy()
    if k % 4 == 0:
        w[:, 0:4] = 0
    return np.ascontiguousarray(w)


def make_in_maps(inputs):
    import ml_dtypes
    BF_NP = ml_dtypes.bfloat16
    x = np.asarray(inputs['x'], np.float32)
    x_tok = np.ascontiguousarray(x.reshape(2048, 1024))
    x_bf = np.ascontiguousarray(x_tok.T.astype(BF_NP))
    x_padT = np.pad(x_tok.T.astype(np.float32), ((0, 0), (4, 0))).astype(BF_NP)
    ipw = (np.asarray(inputs['in_proj_w'], np.float32)
           * np.asarray(inputs['norm_w'], np.float32)[:, None])
    cw = np.asarray(inputs['conv_w'], np.float32)
    cb = np.asarray(inputs['conv_b'], np.float32)
    w_out = np.ascontiguousarray(
        (np.asarray(inputs['ssm_norm_w'], np.float32)[:, None]
         * np.asarray(inputs['out_proj_w'], np.float32)).astype(BF_NP))
    w_sc1 = np.ascontiguousarray(np.asarray(inputs['scorer_w1'], np.float32).astype(BF_NP))
    w_sc2 = np.ascontiguousarray(np.asarray(inputs['scorer_w2'], np.float32).astype(BF_NP))
    w_summ = np.ascontiguousarray(np.asarray(inputs['summ_w'], np.float32).astype(BF_NP))
    w_q = np.ascontiguousarray(np.asarray(inputs['q_w'], np.float32).astype(BF_NP))
    w_k = np.ascontiguousarray(np.asarray(inputs['k_w'], np.float32).astype(BF_NP))
    w_v = np.ascontiguousarray(np.asarray(inputs['v_w'], np.float32).astype(BF_NP))
    w_gate = np.ascontiguousarray(np.asarray(inputs['gate_w'], np.float32).astype(BF_NP))
    in_maps = []
    for k in range(8):
        zc = ipw[:, 256 * k:256 * (k + 1)]
        xc = ipw[:, 2048 + 256 * k:2048 + 256 * (k + 1)]
        bcc = ipw[:, 4096:4352]
        dtc = ipw[:, 4352 + 4 * k:4352 + 4 * (k + 1)]
        w_my = np.ascontiguousarray(
            np.concatenate([zc, xc, bcc, dtc], axis=1).astype(BF_NP))
        conv_rows = np.concatenate([cw[256 * k:256 * (k + 1)], cw[2048:2304]], axis=0)
        convb_rows = np.concatenate([cb[256 * k:256 * (k + 1)], cb[2048:2304]], axis=0)
        m = {
            'x_bf': x_bf,
            'x_bc': _bc_window(x_padT, k),
            'w_in': w_my,
            'conv_w': np.ascontiguousarray(conv_rows),
            'conv_b': np.ascontiguousarray(convb_rows),
            'dt_bias': np.ascontiguousarray(inputs['dt_bias'][4 * k:4 * (k + 1), None]).astype(np.float32),
            'alog32': np.ascontiguousarray(np.repeat(inputs['A_log'][4 * k:4 * (k + 1)], 8)[:, None]).astype(np.float32),
            'd_in': np.ascontiguousarray(
                np.stack([np.repeat(inputs['D'][4 * k:4 * k + 2], 64),
                          np.repeat(inputs['D'][4 * k + 2:4 * k + 4], 64)], axis=1)).astype(np.float32),
            'w_out': w_out,
            'w_sc1': w_sc1,
            'w_sc2': w_sc2,
            'w_summ': w_summ,
            'w_q': w_q,
            'w_k': w_k,
            'w_v': w_v,
            'w_gate': w_gate,
            'x_myT': np.ascontiguousarray(x_tok[256 * k:256 * (k + 1), :].T),
            'tok_ids': np.arange(256 * k, 256 * (k + 1), dtype=np.float32)[:, None],
        }
        in_maps.append(m)
    return in_maps


def gather_out(results):
    out = np.empty((2048, 1024), np.float32)
    for k in range(8):
        out[256 * k:256 * (k + 1), :] = results[k]['out_my'].T
    return out.reshape(2, 1024, 1024)


_CACHED = {}


def _get_nc():
    if "nc" not in _CACHED:
        _CACHED["nc"] = build(upto=9, debug=False)[0]
    return _CACHED["nc"]


def kernel(**inputs):
    from concourse import bass_utils
    nc = _get_nc()
    in_maps = make_in_maps(inputs)
    res = bass_utils.run_bass_kernel_spmd(nc, in_maps, core_ids=list(range(NCORES)))
    return gather_out(res.results)
